# revision 18
# baseline (speedup 1.0000x reference)
"""Canny edge detector on 8 Trainium2 NeuronCores — pure data parallel,
one 1024x1024 image per core.

Per-core pipeline (all decisions in f32, no sqrt/atan2 anywhere):
  gray -> vertical gaussian^2 (PE banded f32 matmuls) -> horizontal
  gaussian (2x 5-tap fused STT passes) -> sobel vertical parts as PE banded
  matmuls, horizontal parts as free-dim taps -> squared-magnitude NMS with
  copy_predicated direction select (row-shifted squared magnitudes via PE
  shift matmuls) -> thresholds on squared magnitude -> 32x bit-packed
  hysteresis flood fill (fused shift/or int passes + stream_shuffle row
  shifts with small DMA boundary fixes).
"""
import math
import numpy as np

B, H, W = 8, 1024, 1024
NB = H // 128          # 8 row blocks
PW = W // 32           # 32 packed words per row per block
PF = NB * PW           # 256 packed words per partition
FF = NB * W            # 8192 f32 elems per partition
N_ITER = 17

_cache = {}


# ---------------------------------------------------------------- constants
def _gauss_v():
    x = np.linspace(-2, 2, 5).astype(np.float64)
    g2 = np.exp(-(x.reshape(5, 1) ** 2 + x.reshape(1, 5) ** 2) / 2.0)
    K = g2 / g2.sum()
    v = K[:, 2] / math.sqrt(K[2, 2])
    return v  # 5-tap 1D gaussian, outer(v,v) = 2D kernel


def _band_matrix(n, taps):
    M = np.zeros((n, n), dtype=np.float64)
    for d, w in taps.items():
        i = np.arange(n)
        j = i + d
        m = (j >= 0) & (j < n)
        M[i[m], j[m]] = w
    return M


def _threshold_sq(t):
    import struct
    t = np.float32(t)

    def f2i(f):
        return struct.unpack('<I', struct.pack('<f', np.float32(f)))[0]

    def i2f(i):
        return np.float32(struct.unpack('<f', struct.pack('<I', i))[0])

    lo_i = f2i(np.float32(0.0))
    hi_i = f2i(np.float32(float(t) * float(t) * 4.0))
    while lo_i + 1 < hi_i:
        mid = (lo_i + hi_i) // 2
        if np.sqrt(i2f(mid), dtype=np.float32) <= t:
            lo_i = mid
        else:
            hi_i = mid
    return i2f(lo_i)


def _build_consts():
    v = _gauss_v()
    Bm = _band_matrix(H, {d - 2: v[d] for d in range(5)})
    BV2 = (Bm @ Bm).astype(np.float32)          # vertical gaussian applied twice
    blocks = []          # list of (t, s)
    mats = []
    for t in range(NB):
        for s in (t - 1, t, t + 1):
            if 0 <= s < NB:
                blk = BV2[128 * t:128 * (t + 1), 128 * s:128 * (s + 1)]
                blocks.append((t, s))
                mats.append(np.ascontiguousarray(blk.T))
    # sobel vertical operators as 128x128 blocks (same for every t)
    sv = np.zeros((128, 128), np.float32)        # s1[p] = g[p-1] + 2 g[p] + g[p+1]
    dv = np.zeros((128, 128), np.float32)        # d1[p] = g[p+1] - g[p-1]
    for p in range(128):
        sv[p, p] = 2.0
        if p > 0:
            sv[p, p - 1] = 1.0
            dv[p, p - 1] = -1.0
        if p < 127:
            sv[p, p + 1] = 1.0
            dv[p, p + 1] = 1.0
    svu = np.zeros((128, 128), np.float32); svu[0, 127] = 1.0     # from block t-1
    svd = np.zeros((128, 128), np.float32); svd[127, 0] = 1.0     # from block t+1
    dvu = np.zeros((128, 128), np.float32); dvu[0, 127] = -1.0
    dvd = np.zeros((128, 128), np.float32); dvd[127, 0] = 1.0
    shup = np.zeros((128, 128), np.float32)      # u[p] = x[p-1]
    shdn = np.zeros((128, 128), np.float32)      # d[p] = x[p+1]
    for p in range(128):
        if p > 0:
            shup[p, p - 1] = 1.0
        if p < 127:
            shdn[p, p + 1] = 1.0
    extra_names = ['sv', 'svu', 'svd', 'dv', 'dvu', 'dvd', 'shup', 'shdn']
    extra_idx = {}
    mats2 = []
    for nm, M in zip(extra_names, [sv, svu, svd, dv, dvu, dvd, shup, shdn]):
        extra_idx[nm] = len(mats2)
        mats2.append(np.ascontiguousarray(M.T))
    bandT = np.ascontiguousarray(np.concatenate(mats, axis=1).astype(np.float32))
    band2T = np.ascontiguousarray(np.concatenate(mats2, axis=1).astype(np.float32))

    # interior mask, packed: bit b of word (p, t*PW + j) is col 32j+b of row 128t+p
    interior = np.zeros((H, W), np.uint32)
    interior[1:-1, 1:-1] = 1
    ip = np.zeros((128, PF), np.uint32)
    for t in range(NB):
        rows = interior[128 * t:128 * (t + 1)]          # [128, W]
        bits = rows.reshape(128, PW, 32)
        words = (bits << np.arange(32, dtype=np.uint32)).sum(axis=2, dtype=np.uint32)
        ip[:, t * PW:(t + 1) * PW] = words
    ip = ip.view(np.int32)

    taps = [np.float32(x) for x in v]      # 5-tap horizontal gaussian
    consts = dict(
        bandT=bandT, band2T=band2T, blocks=blocks, extra_idx=extra_idx,
        interior_packed=np.ascontiguousarray(ip),
        taps=taps,
        KLOW=_threshold_sq(0.1), KHIGH=_threshold_sq(0.2),
        T1SQ=np.float32(np.tan(np.pi / 8) ** 2),
        T2SQ=np.float32(np.tan(3 * np.pi / 8) ** 2),
        zeros_f32=np.zeros((1, W), np.float32),
        zeros_i32=np.zeros((1, PW), np.int32),
    )
    return consts


# ---------------------------------------------------------------- helpers
def _stt(eng, out, in0, scalar, in1, op0, op1):
    from concourse import mybir as mb
    if isinstance(scalar, (int, np.integer)) and not isinstance(scalar, bool):
        imm = mb.ImmediateValue(dtype=mb.dt.int32, value=int(scalar))
    else:
        imm = mb.ImmediateValue(dtype=mb.dt.float32, value=float(scalar))
    return eng.add_instruction(
        mb.InstTensorScalarPtr(
            name=eng.bass.get_next_instruction_name(),
            is_scalar_tensor_tensor=True,
            op0=op0, op1=op1,
            ins=[eng.lower_ap(in0), imm, eng.lower_ap(in1)],
            outs=[eng.lower_ap(out)],
        ))


def _ts_int(eng, out, in0, s0, op0, s1=None, op1=None):
    from concourse import mybir as mb
    ins = [eng.lower_ap(in0), mb.ImmediateValue(dtype=mb.dt.int32, value=int(s0))]
    kw = dict(op0=op0)
    if s1 is not None:
        ins.append(mb.ImmediateValue(dtype=mb.dt.int32, value=int(s1)))
        kw['op1'] = op1
    return eng.add_instruction(
        mb.InstTensorScalarPtr(
            name=eng.bass.get_next_instruction_name(),
            ins=ins,
            outs=[eng.lower_ap(out)],
            **kw,
        ))


# ---------------------------------------------------------------- program
def build_program(debug=False):
    import concourse.tile as tile
    from concourse import bacc, mybir
    from contextlib import ExitStack
    dt = mybir.dt
    op = mybir.AluOpType
    AF = mybir.ActivationFunctionType
    C = _build_consts()
    EI = C['extra_idx']

    nc = bacc.Bacc("TRN2", target_bir_lowering=False, debug=False)
    nblk = C['bandT'].shape[1] // 128
    nblk2 = C['band2T'].shape[1] // 128
    x_d = nc.dram_tensor("x", [3, H, W], dt.float32, kind="ExternalInput").ap()
    band_d = nc.dram_tensor("bandT", [128, nblk * 128], dt.float32, kind="ExternalInput").ap()
    band2_d = nc.dram_tensor("band2T", [128, nblk2 * 128], dt.float32, kind="ExternalInput").ap()
    ip_d = nc.dram_tensor("interior", [128, PF], dt.int32, kind="ExternalInput").ap()
    zf_d = nc.dram_tensor("zeros_f32", [1, W], dt.float32, kind="ExternalInput").ap()
    zi_d = nc.dram_tensor("zeros_i32", [1, PW], dt.int32, kind="ExternalInput").ap()
    out_d = nc.dram_tensor("out", [H, W], dt.float32, kind="ExternalOutput").ap()
    dbg = {}
    if debug:
        for name in ("gray", "g1", "gfull", "gx", "gy", "m2", "kmask", "low0", "e0"):
            dbg[name] = nc.dram_tensor("dbg_" + name, [128, FF], dt.float32,
                                       kind="ExternalOutput").ap()
        for name in ("lowp", "e0p", "ep"):
            dbg[name] = nc.dram_tensor("dbg_" + name, [128, PF], dt.int32,
                                       kind="ExternalOutput").ap()

    with tile.TileContext(nc) as tc, ExitStack() as ctx:
        pool = ctx.enter_context(tc.tile_pool(name="main", bufs=1))
        psA = ctx.enter_context(tc.tile_pool(name="psA", bufs=1, space="PSUM"))
        psB = ctx.enter_context(tc.tile_pool(name="psB", bufs=1, space="PSUM"))

        def f32buf(tag, name):
            return pool.tile([128, FF], dt.float32, tag=tag, name=name)

        def blk(buf, t, n=None):
            if n is None:
                return buf[:, W * t:W * (t + 1)]
            return buf[:, W * t + 512 * n: W * t + 512 * (n + 1)]

        band = pool.tile([128, nblk * 128], dt.float32, tag="Ct", name="band")
        nc.sync.dma_start(band[:], band_d[:])
        band2 = pool.tile([128, nblk2 * 128], dt.float32, tag="band2", name="band2")
        nc.scalar.dma_start(band2[:], band2_d[:])
        ipm = pool.tile([128, PF], dt.int32, tag="ipm", name="ipm")
        nc.scalar.dma_start(ipm[:], ip_d[:])

        def bmat(i):
            return band[:, 128 * i:128 * (i + 1)]

        def bmat2(i):
            return band2[:, 128 * i:128 * (i + 1)]

        # dummy matmul to absorb the const-DMA semaphore on PE early
        dps = psB.tile([128, 128], dt.float32, tag="mmD1", name="dummy", bufs=2)
        nc.tensor.matmul(dps[:], bmat(0), bmat(0), start=True, stop=True)

        # ---------------- gray ---------------------------------------------
        A = f32buf("A", "gray")
        for t in range(NB):
            r = pool.tile([128, W], dt.float32, tag="chR", name="chR")
            g = pool.tile([128, W], dt.float32, tag="chG", name="chG")
            b = pool.tile([128, W], dt.float32, tag="chB", name="chB")
            nc.sync.dma_start(r[:], x_d[0, 128 * t:128 * (t + 1), :])
            nc.scalar.dma_start(g[:], x_d[1, 128 * t:128 * (t + 1), :])
            nc.sync.dma_start(b[:], x_d[2, 128 * t:128 * (t + 1), :])
            sl = blk(A, t)
            nc.scalar.activation(sl, r[:], AF.Copy, scale=0.299)
            _stt(nc.vector, sl, g[:], 0.587, sl, op.mult, op.add)
            _stt(nc.vector, sl, b[:], 0.114, sl, op.mult, op.add)
        if debug:
            nc.sync.dma_start(dbg["gray"][:], A[:])

        # ---------------- vertical gaussian^2 on PE -> Bt -------------------
        Bb = f32buf("Bt", "g1")
        bmap = {}
        for i, (t, s) in enumerate(C['blocks']):
            bmap.setdefault(t, []).append((s, i))
        for t in range(NB):
            for n in range(2):
                ps = psA.tile([128, 512], dt.float32, tag="mmB", name="mmB", bufs=2)
                lst = bmap[t]
                for j, (s, i) in enumerate(lst):
                    nc.tensor.matmul(ps[:], bmat(i), blk(A, s, n),
                                     start=(j == 0), stop=(j == len(lst) - 1))
                nc.scalar.activation(blk(Bb, t, n), ps[:], AF.Copy)
        if debug:
            nc.sync.dma_start(dbg["g1"][:], Bb[:])

        # ---------------- horizontal gaussian (5-tap, twice) ----------------
        taps = C['taps']

        def hconv5(dst, src):
            # out[j] = sum_d v[d] src[j+d-2], zero padded, per 1024-block
            for t in range(NB):
                a = W * t
                ve = nc.vector
                nc.scalar.activation(dst[:, a:a + W], src[:, a:a + W], AF.Copy,
                                     scale=float(taps[2]))
                _stt(ve, dst[:, a + 2:a + W], src[:, a:a + W - 2], float(taps[0]),
                     dst[:, a + 2:a + W], op.mult, op.add)
                _stt(ve, dst[:, a + 1:a + W], src[:, a:a + W - 1], float(taps[1]),
                     dst[:, a + 1:a + W], op.mult, op.add)
                _stt(ve, dst[:, a:a + W - 1], src[:, a + 1:a + W], float(taps[3]),
                     dst[:, a:a + W - 1], op.mult, op.add)
                _stt(ve, dst[:, a:a + W - 2], src[:, a + 2:a + W], float(taps[4]),
                     dst[:, a:a + W - 2], op.mult, op.add)

        hconv5(A, Bb)          # A = BW(g1)   (gray dead)
        hconv5(Bb, A)          # Bb = g_full  (tmp dead)
        if debug:
            nc.sync.dma_start(dbg["gfull"][:], Bb[:])

        # ---------------- sobel vertical parts on PE -------------------------
        # s1 = SV g_full -> Ct ; d1 = DV g_full -> Dt  (banded, corners exact)
        S1 = f32buf("Ct", "s1")
        D1 = f32buf("Dt", "d1")
        for t in range(NB):
            for n in range(2):
                ps = psA.tile([128, 512], dt.float32, tag="mmS", name="mmS", bufs=2)
                parts = [(EI['sv'], t)]
                if t > 0:
                    parts.append((EI['svu'], t - 1))
                if t < NB - 1:
                    parts.append((EI['svd'], t + 1))
                for j, (mi, s) in enumerate(parts):
                    nc.tensor.matmul(ps[:], bmat2(mi), blk(Bb, s, n),
                                     start=(j == 0), stop=(j == len(parts) - 1))
                nc.scalar.activation(blk(S1, t, n), ps[:], AF.Copy)
                ps2 = psB.tile([128, 512], dt.float32, tag="mmD1", name="mmD1", bufs=2)
                parts = [(EI['dv'], t)]
                if t > 0:
                    parts.append((EI['dvu'], t - 1))
                if t < NB - 1:
                    parts.append((EI['dvd'], t + 1))
                for j, (mi, s) in enumerate(parts):
                    nc.tensor.matmul(ps2[:], bmat2(mi), blk(Bb, s, n),
                                     start=(j == 0), stop=(j == len(parts) - 1))
                nc.scalar.activation(blk(D1, t, n), ps2[:], AF.Copy)

        # gx = D_W(s1) -> A ; gy = S_W(d1) -> Et
        GX = A
        GY = f32buf("Et", "gy")
        for t in range(NB):
            a = W * t
            ve = nc.vector
            # gx[j] = s1[j+1] - s1[j-1]
            nc.scalar.activation(GX[:, a:a + W - 1], S1[:, a + 1:a + W], AF.Copy)
            nc.vector.memset(GX[:, a + W - 1:a + W], 0.0)
            _stt(ve, GX[:, a + 1:a + W], S1[:, a:a + W - 1], -1.0,
                 GX[:, a + 1:a + W], op.mult, op.add)
            # gy[j] = d1[j-1] + 2 d1[j] + d1[j+1]
            nc.scalar.activation(GY[:, a:a + W], D1[:, a:a + W], AF.Copy, scale=2.0)
            _stt(ve, GY[:, a + 1:a + W], D1[:, a:a + W - 1], 1.0,
                 GY[:, a + 1:a + W], op.mult, op.add)
            _stt(ve, GY[:, a:a + W - 1], D1[:, a + 1:a + W], 1.0,
                 GY[:, a:a + W - 1], op.mult, op.add)
        if debug:
            nc.sync.dma_start(dbg["gx"][:], GX[:])
            nc.sync.dma_start(dbg["gy"][:], GY[:])

        # ---------------- m2 / direction masks ------------------------------
        M2X = Bb            # gfull dead
        M2Y = D1            # d1 dead after gy
        nc.scalar.activation(M2X[:], GX[:], AF.Square)
        nc.scalar.activation(M2Y[:], GY[:], AF.Square)
        c0 = pool.tile([128, FF], dt.uint8, tag="haa", name="c0")
        c2 = pool.tile([128, FF], dt.uint8, tag="hcc", name="c2")
        c1 = pool.tile([128, FF], dt.uint8, tag="hbb", name="c1")
        _stt(nc.vector, c0[:], M2X[:], float(C['T1SQ']), M2Y[:], op.mult, op.is_ge)
        _stt(nc.vector, c2[:], M2X[:], float(C['T2SQ']), M2Y[:], op.mult, op.is_le)
        M2 = S1             # s1 dead after gx
        nc.gpsimd.tensor_tensor(M2[:], M2X[:], M2Y[:], op.add)
        PROD = GX           # overwrite gx in place
        nc.vector.tensor_tensor(PROD[:], GX[:], GY[:], op.mult)
        # c1 = (prod >= 0) & !(c0 | c2)
        nc.vector.tensor_tensor(c1[:], c0[:], c2[:], op.logical_or)
        nc.vector.tensor_scalar(c1[:], c1[:], 0.0, None, op.is_equal)
        _stt(nc.vector, c1[:], PROD[:], 0.0, c1[:], op.is_ge, op.logical_and)
        if debug:
            nc.sync.dma_start(dbg["m2"][:], M2[:])

        # ---------------- NMS ------------------------------------------------
        # m2u = m2[row-1] -> M2X slot ; m2d = m2[row+1] -> M2Y slot (PE shifts)
        M2U = Bb
        M2D = D1
        for t in range(NB):
            for n in range(2):
                ps = psA.tile([128, 512], dt.float32, tag="mmU", name="mmU", bufs=1)
                nc.tensor.matmul(ps[:], bmat2(EI['shup']), blk(M2, t, n),
                                 start=True, stop=True)
                nc.scalar.activation(blk(M2U, t, n), ps[:], AF.Copy)
                ps2 = psB.tile([128, 512], dt.float32, tag="mmV", name="mmV", bufs=1)
                nc.tensor.matmul(ps2[:], bmat2(EI['shdn']), blk(M2, t, n),
                                 start=True, stop=True)
                nc.scalar.activation(blk(M2D, t, n), ps2[:], AF.Copy)
        # boundary rows across blocks
        nc.sync.dma_start(M2U[0:1, W:FF], M2[127:128, 0:FF - W])
        nc.vector.memset(M2U[0:1, 0:W], 0.0)
        nc.scalar.dma_start(M2D[127:128, 0:FF - W], M2[0:1, W:FF])
        nc.scalar.dma_start(M2D[127:128, FF - W:FF], zf_d[:])

        km = GY             # gy dead after prod
        scrtags = ["chR", "chG", "chB"]

        def nms_cat(catmask, a1, sh1, a2, sh2, first, ci):
            for t in range(NB):
                a = W * t
                lo = max(0, -sh1, -sh2)
                hi = min(W, W - sh1, W - sh2)
                tg = scrtags[(ci * NB + t) % 3]
                eng = nc.vector
                scr = pool.tile([128, W], dt.float32, tag=tg, name="scr")
                eng.tensor_tensor(scr[:, lo:hi],
                                  a1[:, a + lo + sh1:a + hi + sh1],
                                  a2[:, a + lo + sh2:a + hi + sh2], op.max)
                eng.tensor_tensor(scr[:, lo:hi], M2[:, a + lo:a + hi],
                                  scr[:, lo:hi], op.is_ge)
                if first:
                    nc.vector.tensor_copy(km[:, a + lo:a + hi], scr[:, lo:hi])
                    if lo > 0:
                        nc.vector.memset(km[:, a:a + lo], 0.0)
                        nc.vector.memset(km[:, a + hi:a + W], 0.0)
                else:
                    nc.vector.copy_predicated(km[:, a + lo:a + hi],
                                              catmask[:, a + lo:a + hi],
                                              scr[:, lo:hi])

        nms_cat(None, M2U, -1, M2D, 1, True, 0)     # cat3
        nms_cat(c2, M2U, 0, M2D, 0, False, 1)       # cat2
        nms_cat(c1, M2U, 1, M2D, -1, False, 2)      # cat1
        nms_cat(c0, M2, -1, M2, 1, False, 3)        # cat0
        if debug:
            nc.sync.dma_start(dbg["kmask"][:], km[:])

        # ---------------- thresholds -> f32 0/1 ------------------------------
        lowf = GX            # prod dead
        e0f = M2D            # m2d dead after cat1
        _stt(nc.vector, lowf[:], M2[:], float(C['KLOW']), km[:], op.is_gt, op.mult)
        _stt(nc.vector, e0f[:], M2[:], float(C['KHIGH']), km[:], op.is_gt, op.mult)
        if debug:
            nc.sync.dma_start(dbg["low0"][:], lowf[:])
            nc.sync.dma_start(dbg["e0"][:], e0f[:])

        # ---------------- pack to bits ---------------------------------------
        pia = pool.tile([128, FF], dt.int32, tag="Ct", name="pia")
        pib = pool.tile([128, FF], dt.int32, tag="Et", name="pib")

        def pack(dstp, srcf):
            nc.vector.tensor_copy(pia[:], srcf[:])          # f32 0/1 -> int32
            cur = pia[:]
            n = FF
            for lvl in range(5):
                n //= 2
                sh = 1 << lvl
                vw = cur.rearrange("p (n two) -> p n two", two=2)
                if lvl == 4:
                    dst = dstp[:]
                elif lvl % 2 == 0:
                    dst = pib[:, 0:n]
                else:
                    dst = pia[:, 0:n]
                _stt(nc.vector, dst, vw[:, :, 1], sh, vw[:, :, 0],
                     op.logical_shift_left, op.bitwise_or)
                cur = dst

        lowp = pool.tile([128, PF], dt.int32, tag="lp2", name="lowp")
        e0p = pool.tile([128, PF], dt.int32, tag="ep2", name="e0p")
        pack(lowp, lowf)
        pack(e0p, e0f)
        nc.vector.tensor_tensor(lowp[:], lowp[:], ipm[:], op.bitwise_and)
        nc.vector.tensor_tensor(e0p[:], e0p[:], ipm[:], op.bitwise_and)
        if debug:
            nc.sync.dma_start(dbg["lowp"][:], lowp[:])
            nc.sync.dma_start(dbg["e0p"][:], e0p[:])

        # ---------------- hysteresis flood fill ------------------------------
        e = e0p
        aa = pool.tile([128, PF], dt.int32, tag="haa", name="haa")
        bb2 = pool.tile([128, PF], dt.int32, tag="hbb", name="hbb")
        cc = pool.tile([128, PF], dt.int32, tag="hcc", name="hcc")
        dup = pool.tile([128, PF], dt.int32, tag="hdup", name="hdup")
        ddn = pool.tile([128, PF], dt.int32, tag="hddn", name="hddn")
        av = aa.rearrange("p (n w) -> p n w", w=PW)
        bv = bb2.rearrange("p (n w) -> p n w", w=PW)
        cv = cc.rearrange("p (n w) -> p n w", w=PW)
        mask_up = [min(i + 1, 31) for i in range(32)]   # dup[p] = c[p+1]
        mask_dn = [max(i - 1, 0) for i in range(32)]    # ddn[p] = c[p-1]
        for it in range(N_ITER):
            # horizontal dilate with cross-word carries
            _stt(nc.vector, aa[:], e[:], 1, e[:], op.logical_shift_left, op.bitwise_or)
            _stt(nc.vector, aa[:], e[:], 1, aa[:], op.logical_shift_right, op.bitwise_or)
            _stt(nc.vector, bb2[:, 1:PF], e[:, 0:PF - 1], 31, aa[:, 1:PF],
                 op.logical_shift_right, op.bitwise_or)
            nc.vector.tensor_copy(bb2[:, 0:1], aa[:, 0:1])
            nc.vector.tensor_copy(bv[:, :, 0], av[:, :, 0])
            _stt(nc.vector, cc[:, 0:PF - 1], e[:, 1:PF], 31, bb2[:, 0:PF - 1],
                 op.logical_shift_left, op.bitwise_or)
            nc.vector.tensor_copy(cc[:, PF - 1:PF], bb2[:, PF - 1:PF])
            nc.vector.tensor_copy(cv[:, :, PW - 1], bv[:, :, PW - 1])
            # vertical neighbors via stream_shuffle + boundary DMAs
            nc.vector.stream_shuffle(dup[:], cc[:], mask_up)
            nc.vector.stream_shuffle(ddn[:], cc[:], mask_dn)
            q1 = nc.sync if it % 2 == 0 else nc.scalar
            q2 = nc.scalar if it % 2 == 0 else nc.sync
            # rows masked to zero by lowp's interior mask need no zero fill
            q1.dma_start(dup[31:127:32, :], cc[32:128:32, :])
            q1.dma_start(dup[127:128, 0:PF - PW], cc[0:1, PW:PF])
            q2.dma_start(ddn[32:128:32, :], cc[31:127:32, :])
            q2.dma_start(ddn[0:1, PW:PF], cc[127:128, 0:PF - PW])
            # e' = (dup | ddn | c) & low
            nc.vector.tensor_tensor(dup[:], dup[:], ddn[:], op.bitwise_or)
            nc.vector.tensor_tensor(dup[:], dup[:], cc[:], op.bitwise_or)
            nc.vector.tensor_tensor(e[:], dup[:], lowp[:], op.bitwise_and)
        if debug:
            nc.sync.dma_start(dbg["ep"][:], e[:])

        # ---------------- unpack complement -> output ------------------------
        ne = aa
        _stt(nc.vector, ne[:], e[:], -1, e[:], op.bitwise_xor, op.bypass)
        outf = f32buf("A", "outf")
        cur = ne[:]
        n = PF
        masks = [0xFFFF, 0xFF, 0xF, 0x3, 0x1]
        for lvl in range(5):
            half = 32 >> (lvl + 1)       # bits per half-word after split
            m = masks[lvl]
            dst = (pib[:, 0:2 * n] if lvl % 2 == 0 else pia[:, 0:2 * n])
            dv2 = dst.rearrange("p (n two) -> p n two", two=2)
            _ts_int(nc.vector, dv2[:, :, 0], cur, m, op.bitwise_and)
            _ts_int(nc.vector, dv2[:, :, 1], cur, half, op.logical_shift_right,
                    m, op.bitwise_and)
            cur = dst
            n *= 2
        nc.vector.tensor_copy(outf[:], pib[:])    # int 0/1 -> f32
        for t in range(NB):
            q = nc.sync if t % 2 == 0 else nc.scalar
            q.dma_start(out_d[128 * t:128 * (t + 1), :], outf[:, W * t:W * (t + 1)])

    nc.compile()
    return nc, C, dbg


def _run(inputs, debug=False, trace=False):
    from concourse.bass_utils import run_bass_kernel_spmd
    key = ("dbg" if debug else "plain")
    if key not in _cache:
        _cache[key] = build_program(debug=debug)
    nc, C, dbg = _cache[key]
    x = np.asarray(inputs["x"], dtype=np.float32)
    in_maps = []
    for c in range(B):
        in_maps.append({
            "x": np.ascontiguousarray(x[c]),
            "bandT": C['bandT'],
            "band2T": C['band2T'],
            "interior": C['interior_packed'],
            "zeros_f32": C['zeros_f32'],
            "zeros_i32": C['zeros_i32'],
        })
    res = run_bass_kernel_spmd(nc, in_maps, core_ids=list(range(B)), trace=trace)
    return res


def kernel(x, gaussian_kernel=None, sobel_x=None, sobel_y=None):
    res = _run({"x": x})
    out = np.stack([res.results[c]["out"] for c in range(B)], axis=0)
    return out.reshape(B, 1, H, W).astype(np.float32)


# revision 19
# speedup vs baseline: 1.0624x; 1.0624x over previous
"""Canny edge detector on 8 Trainium2 NeuronCores — pure data parallel,
one 1024x1024 image per core.

Per-core pipeline (all decisions in f32, no sqrt/atan2 anywhere):
  gray -> vertical gaussian^2 (PE banded f32 matmuls) -> horizontal
  gaussian (2x 5-tap fused STT passes) -> sobel vertical parts as PE banded
  matmuls, horizontal parts as free-dim taps -> squared-magnitude NMS with
  copy_predicated direction select (row-shifted squared magnitudes via PE
  shift matmuls) -> thresholds on squared magnitude -> 32x bit-packed
  hysteresis flood fill (fused shift/or int passes + stream_shuffle row
  shifts with small DMA boundary fixes).
"""
import math
import numpy as np

B, H, W = 8, 1024, 1024
NB = H // 128          # 8 row blocks
PW = W // 32           # 32 packed words per row per block
PF = NB * PW           # 256 packed words per partition
FF = NB * W            # 8192 f32 elems per partition
N_ITER = 17

_cache = {}


# ---------------------------------------------------------------- constants
def _gauss_v():
    x = np.linspace(-2, 2, 5).astype(np.float64)
    g2 = np.exp(-(x.reshape(5, 1) ** 2 + x.reshape(1, 5) ** 2) / 2.0)
    K = g2 / g2.sum()
    v = K[:, 2] / math.sqrt(K[2, 2])
    return v  # 5-tap 1D gaussian, outer(v,v) = 2D kernel


def _band_matrix(n, taps):
    M = np.zeros((n, n), dtype=np.float64)
    for d, w in taps.items():
        i = np.arange(n)
        j = i + d
        m = (j >= 0) & (j < n)
        M[i[m], j[m]] = w
    return M


def _threshold_sq(t):
    import struct
    t = np.float32(t)

    def f2i(f):
        return struct.unpack('<I', struct.pack('<f', np.float32(f)))[0]

    def i2f(i):
        return np.float32(struct.unpack('<f', struct.pack('<I', i))[0])

    lo_i = f2i(np.float32(0.0))
    hi_i = f2i(np.float32(float(t) * float(t) * 4.0))
    while lo_i + 1 < hi_i:
        mid = (lo_i + hi_i) // 2
        if np.sqrt(i2f(mid), dtype=np.float32) <= t:
            lo_i = mid
        else:
            hi_i = mid
    return i2f(lo_i)


def _build_consts():
    v = _gauss_v()
    Bm = _band_matrix(H, {d - 2: v[d] for d in range(5)})
    BV2 = (Bm @ Bm).astype(np.float32)          # vertical gaussian applied twice
    blocks = []          # list of (t, s)
    mats = []
    for t in range(NB):
        for s in (t - 1, t, t + 1):
            if 0 <= s < NB:
                blk = BV2[128 * t:128 * (t + 1), 128 * s:128 * (s + 1)]
                blocks.append((t, s))
                mats.append(np.ascontiguousarray(blk.T))
    # sobel vertical operators as 128x128 blocks (same for every t)
    sv = np.zeros((128, 128), np.float32)        # s1[p] = g[p-1] + 2 g[p] + g[p+1]
    dv = np.zeros((128, 128), np.float32)        # d1[p] = g[p+1] - g[p-1]
    for p in range(128):
        sv[p, p] = 2.0
        if p > 0:
            sv[p, p - 1] = 1.0
            dv[p, p - 1] = -1.0
        if p < 127:
            sv[p, p + 1] = 1.0
            dv[p, p + 1] = 1.0
    svu = np.zeros((128, 128), np.float32); svu[0, 127] = 1.0     # from block t-1
    svd = np.zeros((128, 128), np.float32); svd[127, 0] = 1.0     # from block t+1
    dvu = np.zeros((128, 128), np.float32); dvu[0, 127] = -1.0
    dvd = np.zeros((128, 128), np.float32); dvd[127, 0] = 1.0
    shup = np.zeros((128, 128), np.float32)      # u[p] = x[p-1]
    shdn = np.zeros((128, 128), np.float32)      # d[p] = x[p+1]
    for p in range(128):
        if p > 0:
            shup[p, p - 1] = 1.0
        if p < 127:
            shdn[p, p + 1] = 1.0
    ident = np.eye(128, dtype=np.float32)
    extra_names = ['sv', 'svu', 'svd', 'dv', 'dvu', 'dvd', 'shup', 'shdn', 'ident']
    extra_idx = {}
    mats2 = []
    for nm, M in zip(extra_names, [sv, svu, svd, dv, dvu, dvd, shup, shdn, ident]):
        extra_idx[nm] = len(mats2)
        mats2.append(np.ascontiguousarray(M.T))
    bandT = np.ascontiguousarray(np.concatenate(mats, axis=1).astype(np.float32))
    band2T = np.ascontiguousarray(np.concatenate(mats2, axis=1).astype(np.float32))

    # horizontal gaussian^2 as matrix product C = Bw @ Bw (exact border rows/cols),
    # stored as a Toeplitz master strip + exact first/last block columns
    C64 = Bm @ Bm
    Cf = C64.astype(np.float32)
    w9 = np.array([C64[512, 512 + k - 4] for k in range(9)])
    masterS = np.zeros((128, 1152), np.float64)
    for p in range(128):
        lo = max(0, 512 + p - 4)
        for j in range(lo, min(1152, 512 + p + 5)):
            masterS[p, j] = w9[j - 512 - p + 4]
    masterS = masterS.astype(np.float32)
    for n in range(2):
        for vv in ([0, 1, 2, 3, 4] if n == 0 else [3, 4, 5, 6, 7]):
            if (vv, n) in [(0, 0), (7, 1)]:
                continue
            o = 512 + 512 * n - 128 * vv
            assert (Cf[128 * vv:128 * vv + 128, 512 * n:512 * n + 512]
                    == masterS[:, o:o + 512]).all()
    hcC = np.concatenate([masterS, Cf[0:128, 0:512], Cf[896:1024, 512:1024]],
                         axis=1)
    hcC = np.ascontiguousarray(hcC.astype(np.float32))

    # interior mask, packed: bit b of word (p, t*PW + j) is col 32j+b of row 128t+p
    interior = np.zeros((H, W), np.uint32)
    interior[1:-1, 1:-1] = 1
    ip = np.zeros((128, PF), np.uint32)
    for t in range(NB):
        rows = interior[128 * t:128 * (t + 1)]          # [128, W]
        bits = rows.reshape(128, PW, 32)
        words = (bits << np.arange(32, dtype=np.uint32)).sum(axis=2, dtype=np.uint32)
        ip[:, t * PW:(t + 1) * PW] = words
    ip = ip.view(np.int32)

    taps = [np.float32(x) for x in v]      # 5-tap horizontal gaussian
    consts = dict(
        bandT=bandT, band2T=band2T, hcC=hcC, blocks=blocks, extra_idx=extra_idx,
        interior_packed=np.ascontiguousarray(ip),
        taps=taps,
        KLOW=_threshold_sq(0.1), KHIGH=_threshold_sq(0.2),
        T1SQ=np.float32(np.tan(np.pi / 8) ** 2),
        T2SQ=np.float32(np.tan(3 * np.pi / 8) ** 2),
        zeros_f32=np.zeros((1, W), np.float32),
        zeros_i32=np.zeros((1, PW), np.int32),
    )
    return consts


# ---------------------------------------------------------------- helpers
def _stt(eng, out, in0, scalar, in1, op0, op1):
    from concourse import mybir as mb
    if isinstance(scalar, (int, np.integer)) and not isinstance(scalar, bool):
        imm = mb.ImmediateValue(dtype=mb.dt.int32, value=int(scalar))
    else:
        imm = mb.ImmediateValue(dtype=mb.dt.float32, value=float(scalar))
    return eng.add_instruction(
        mb.InstTensorScalarPtr(
            name=eng.bass.get_next_instruction_name(),
            is_scalar_tensor_tensor=True,
            op0=op0, op1=op1,
            ins=[eng.lower_ap(in0), imm, eng.lower_ap(in1)],
            outs=[eng.lower_ap(out)],
        ))


def _ts_int(eng, out, in0, s0, op0, s1=None, op1=None):
    from concourse import mybir as mb
    ins = [eng.lower_ap(in0), mb.ImmediateValue(dtype=mb.dt.int32, value=int(s0))]
    kw = dict(op0=op0)
    if s1 is not None:
        ins.append(mb.ImmediateValue(dtype=mb.dt.int32, value=int(s1)))
        kw['op1'] = op1
    return eng.add_instruction(
        mb.InstTensorScalarPtr(
            name=eng.bass.get_next_instruction_name(),
            ins=ins,
            outs=[eng.lower_ap(out)],
            **kw,
        ))


# ---------------------------------------------------------------- program
def build_program(debug=False):
    import concourse.tile as tile
    from concourse import bacc, mybir
    from contextlib import ExitStack
    dt = mybir.dt
    op = mybir.AluOpType
    AF = mybir.ActivationFunctionType
    C = _build_consts()
    EI = C['extra_idx']

    nc = bacc.Bacc("TRN2", target_bir_lowering=False, debug=False)
    nblk = C['bandT'].shape[1] // 128
    nblk2 = C['band2T'].shape[1] // 128
    x_d = nc.dram_tensor("x", [3, H, W], dt.float32, kind="ExternalInput").ap()
    band_d = nc.dram_tensor("bandT", [128, nblk * 128], dt.float32, kind="ExternalInput").ap()
    band2_d = nc.dram_tensor("band2T", [128, nblk2 * 128], dt.float32, kind="ExternalInput").ap()
    hc_d = nc.dram_tensor("hcC", [128, 2176], dt.float32, kind="ExternalInput").ap()
    ip_d = nc.dram_tensor("interior", [128, PF], dt.int32, kind="ExternalInput").ap()
    zf_d = nc.dram_tensor("zeros_f32", [1, W], dt.float32, kind="ExternalInput").ap()
    zi_d = nc.dram_tensor("zeros_i32", [1, PW], dt.int32, kind="ExternalInput").ap()
    out_d = nc.dram_tensor("out", [H, W], dt.float32, kind="ExternalOutput").ap()
    dbg = {}
    if debug:
        for name in ("gray", "g1", "gfull", "gx", "gy", "m2", "kmask", "low0", "e0"):
            dbg[name] = nc.dram_tensor("dbg_" + name, [128, FF], dt.float32,
                                       kind="ExternalOutput").ap()
        for name in ("lowp", "e0p", "ep"):
            dbg[name] = nc.dram_tensor("dbg_" + name, [128, PF], dt.int32,
                                       kind="ExternalOutput").ap()

    with tile.TileContext(nc) as tc, ExitStack() as ctx:
        pool = ctx.enter_context(tc.tile_pool(name="main", bufs=1))
        psA = ctx.enter_context(tc.tile_pool(name="psA", bufs=1, space="PSUM"))
        psB = ctx.enter_context(tc.tile_pool(name="psB", bufs=1, space="PSUM"))

        def f32buf(tag, name):
            return pool.tile([128, FF], dt.float32, tag=tag, name=name)

        def blk(buf, t, n=None):
            if n is None:
                return buf[:, W * t:W * (t + 1)]
            return buf[:, W * t + 512 * n: W * t + 512 * (n + 1)]

        band = pool.tile([128, nblk * 128], dt.float32, tag="Ct", name="band")
        nc.sync.dma_start(band[:], band_d[:])
        band2 = pool.tile([128, nblk2 * 128], dt.float32, tag="band2", name="band2")
        nc.scalar.dma_start(band2[:], band2_d[:])
        ipm = pool.tile([128, PF], dt.int32, tag="ipm", name="ipm")
        nc.scalar.dma_start(ipm[:], ip_d[:])

        def bmat(i):
            return band[:, 128 * i:128 * (i + 1)]

        def bmat2(i):
            return band2[:, 128 * i:128 * (i + 1)]

        # dummy matmul to absorb the const-DMA semaphore on PE early
        dps = psB.tile([128, 128], dt.float32, tag="mmD1", name="dummy", bufs=2)
        nc.tensor.matmul(dps[:], bmat(0), bmat(0), start=True, stop=True)

        # ---------------- gray ---------------------------------------------
        A = f32buf("A", "gray")
        for t in range(NB):
            r = pool.tile([128, W], dt.float32, tag="chR", name="chR")
            g = pool.tile([128, W], dt.float32, tag="chG", name="chG")
            b = pool.tile([128, W], dt.float32, tag="chB", name="chB")
            nc.sync.dma_start(r[:], x_d[0, 128 * t:128 * (t + 1), :])
            nc.scalar.dma_start(g[:], x_d[1, 128 * t:128 * (t + 1), :])
            nc.sync.dma_start(b[:], x_d[2, 128 * t:128 * (t + 1), :])
            sl = blk(A, t)
            nc.scalar.activation(sl, r[:], AF.Copy, scale=0.299)
            _stt(nc.vector, sl, g[:], 0.587, sl, op.mult, op.add)
            _stt(nc.vector, sl, b[:], 0.114, sl, op.mult, op.add)
        if debug:
            nc.sync.dma_start(dbg["gray"][:], A[:])

        # ---------------- vertical gaussian^2 on PE -> Bt -------------------
        Bb = f32buf("Bt", "g1")
        bmap = {}
        for i, (t, s) in enumerate(C['blocks']):
            bmap.setdefault(t, []).append((s, i))
        for t in range(NB):
            for n in range(2):
                ps = psA.tile([128, 512], dt.float32, tag="mmB", name="mmB", bufs=2)
                lst = bmap[t]
                for j, (s, i) in enumerate(lst):
                    nc.tensor.matmul(ps[:], bmat(i), blk(A, s, n),
                                     start=(j == 0), stop=(j == len(lst) - 1))
                nc.scalar.activation(blk(Bb, t, n), ps[:], AF.Copy)
        if debug:
            nc.sync.dma_start(dbg["g1"][:], Bb[:])

        # ---------------- horizontal gaussian (5-tap, twice) ----------------
        taps = C['taps']

        def hconv5(dst, src):
            # out[j] = sum_d v[d] src[j+d-2], zero padded, per 1024-block
            for t in range(NB):
                a = W * t
                ve = nc.vector
                nc.scalar.activation(dst[:, a:a + W], src[:, a:a + W], AF.Copy,
                                     scale=float(taps[2]))
                _stt(ve, dst[:, a + 2:a + W], src[:, a:a + W - 2], float(taps[0]),
                     dst[:, a + 2:a + W], op.mult, op.add)
                _stt(ve, dst[:, a + 1:a + W], src[:, a:a + W - 1], float(taps[1]),
                     dst[:, a + 1:a + W], op.mult, op.add)
                _stt(ve, dst[:, a:a + W - 1], src[:, a + 1:a + W], float(taps[3]),
                     dst[:, a:a + W - 1], op.mult, op.add)
                _stt(ve, dst[:, a:a + W - 2], src[:, a + 2:a + W], float(taps[4]),
                     dst[:, a:a + W - 2], op.mult, op.add)

        # g_full = g1 @ C on PE: per row-tile, transpose 128x128 subtiles of g1
        # (PE transpose mode), then banded matmuls against the C master strip.
        hcC = pool.tile([128, 2176], dt.float32, tag="Dt", name="hcC")
        nc.sync.dma_start(hcC[:], hc_d[:])
        GF = A              # gray dead; g_full lands in A
        for t in range(NB):
            g1T = pool.tile([128, 1024], dt.float32, tag="chR", name="g1T")
            for k in range(8):
                pst = psB.tile([128, 128], dt.float32, tag="mmD1", name="tp", bufs=2)
                nc.tensor.transpose(pst[:], Bb[:, W * t + 128 * k: W * t + 128 * (k + 1)],
                                    bmat2(EI['ident']))
                nc.scalar.activation(g1T[:, 128 * k:128 * (k + 1)], pst[:], AF.Copy)
            for n in range(2):
                pso = psA.tile([128, 512], dt.float32, tag="mmB", name="hco", bufs=2)
                vs = [0, 1, 2, 3, 4] if n == 0 else [3, 4, 5, 6, 7]
                for j, vv in enumerate(vs):
                    if (vv, n) == (0, 0):
                        rhs = hcC[:, 1152:1664]
                    elif (vv, n) == (7, 1):
                        rhs = hcC[:, 1664:2176]
                    else:
                        o = 512 + 512 * n - 128 * vv
                        rhs = hcC[:, o:o + 512]
                    nc.tensor.matmul(pso[:], g1T[:, 128 * vv:128 * (vv + 1)], rhs,
                                     start=(j == 0), stop=(j == len(vs) - 1))
                nc.scalar.activation(blk(GF, t, n), pso[:], AF.Copy)
        if debug:
            nc.sync.dma_start(dbg["gfull"][:], GF[:])

        # ---------------- sobel vertical parts on PE -------------------------
        # s1 = SV g_full -> Ct ; d1 = DV g_full -> Dt  (banded, corners exact)
        S1 = f32buf("Ct", "s1")
        D1 = f32buf("Dt", "d1")
        for t in range(NB):
            for n in range(2):
                ps = psA.tile([128, 512], dt.float32, tag="mmS", name="mmS", bufs=2)
                parts = [(EI['sv'], t)]
                if t > 0:
                    parts.append((EI['svu'], t - 1))
                if t < NB - 1:
                    parts.append((EI['svd'], t + 1))
                for j, (mi, s) in enumerate(parts):
                    nc.tensor.matmul(ps[:], bmat2(mi), blk(GF, s, n),
                                     start=(j == 0), stop=(j == len(parts) - 1))
                nc.scalar.activation(blk(S1, t, n), ps[:], AF.Copy)
                ps2 = psB.tile([128, 512], dt.float32, tag="mmD1", name="mmD1", bufs=2)
                parts = [(EI['dv'], t)]
                if t > 0:
                    parts.append((EI['dvu'], t - 1))
                if t < NB - 1:
                    parts.append((EI['dvd'], t + 1))
                for j, (mi, s) in enumerate(parts):
                    nc.tensor.matmul(ps2[:], bmat2(mi), blk(GF, s, n),
                                     start=(j == 0), stop=(j == len(parts) - 1))
                nc.scalar.activation(blk(D1, t, n), ps2[:], AF.Copy)

        # gx = D_W(s1) -> Bb ; gy = S_W(d1) -> Et
        GX = Bb
        GY = f32buf("Et", "gy")
        for t in range(NB):
            a = W * t
            ve = nc.vector
            # gx[j] = s1[j+1] - s1[j-1]
            nc.scalar.activation(GX[:, a:a + W - 1], S1[:, a + 1:a + W], AF.Copy)
            nc.vector.memset(GX[:, a + W - 1:a + W], 0.0)
            _stt(ve, GX[:, a + 1:a + W], S1[:, a:a + W - 1], -1.0,
                 GX[:, a + 1:a + W], op.mult, op.add)
            # gy[j] = d1[j-1] + 2 d1[j] + d1[j+1]
            nc.scalar.activation(GY[:, a:a + W], D1[:, a:a + W], AF.Copy, scale=2.0)
            _stt(ve, GY[:, a + 1:a + W], D1[:, a:a + W - 1], 1.0,
                 GY[:, a + 1:a + W], op.mult, op.add)
            _stt(ve, GY[:, a:a + W - 1], D1[:, a + 1:a + W], 1.0,
                 GY[:, a:a + W - 1], op.mult, op.add)
        if debug:
            nc.sync.dma_start(dbg["gx"][:], GX[:])
            nc.sync.dma_start(dbg["gy"][:], GY[:])

        # ---------------- m2 / direction masks ------------------------------
        M2X = A             # g_full dead after sobel PE
        M2Y = D1            # d1 dead after gy
        nc.scalar.activation(M2X[:], GX[:], AF.Square)
        nc.scalar.activation(M2Y[:], GY[:], AF.Square)
        c0 = pool.tile([128, FF], dt.uint8, tag="haa", name="c0")
        c2 = pool.tile([128, FF], dt.uint8, tag="hcc", name="c2")
        c1 = pool.tile([128, FF], dt.uint8, tag="hbb", name="c1")
        _stt(nc.vector, c0[:], M2X[:], float(C['T1SQ']), M2Y[:], op.mult, op.is_ge)
        _stt(nc.vector, c2[:], M2X[:], float(C['T2SQ']), M2Y[:], op.mult, op.is_le)
        M2 = S1             # s1 dead after gx
        nc.gpsimd.tensor_tensor(M2[:], M2X[:], M2Y[:], op.add)
        PROD = GX           # overwrite gx in place
        nc.vector.tensor_tensor(PROD[:], GX[:], GY[:], op.mult)
        # c1 = (prod >= 0) & !(c0 | c2)
        nc.vector.tensor_tensor(c1[:], c0[:], c2[:], op.logical_or)
        nc.vector.tensor_scalar(c1[:], c1[:], 0.0, None, op.is_equal)
        _stt(nc.vector, c1[:], PROD[:], 0.0, c1[:], op.is_ge, op.logical_and)
        if debug:
            nc.sync.dma_start(dbg["m2"][:], M2[:])

        # ---------------- NMS ------------------------------------------------
        # m2u = m2[row-1] -> M2X slot ; m2d = m2[row+1] -> M2Y slot (PE shifts)
        M2U = A
        M2D = D1
        for t in range(NB):
            for n in range(2):
                ps = psA.tile([128, 512], dt.float32, tag="mmU", name="mmU", bufs=1)
                nc.tensor.matmul(ps[:], bmat2(EI['shup']), blk(M2, t, n),
                                 start=True, stop=True)
                nc.scalar.activation(blk(M2U, t, n), ps[:], AF.Copy)
                ps2 = psB.tile([128, 512], dt.float32, tag="mmV", name="mmV", bufs=1)
                nc.tensor.matmul(ps2[:], bmat2(EI['shdn']), blk(M2, t, n),
                                 start=True, stop=True)
                nc.scalar.activation(blk(M2D, t, n), ps2[:], AF.Copy)
        # boundary rows across blocks
        nc.sync.dma_start(M2U[0:1, W:FF], M2[127:128, 0:FF - W])
        nc.vector.memset(M2U[0:1, 0:W], 0.0)
        nc.scalar.dma_start(M2D[127:128, 0:FF - W], M2[0:1, W:FF])
        nc.scalar.dma_start(M2D[127:128, FF - W:FF], zf_d[:])

        km = GY             # gy dead after prod
        scrtags = ["chR", "chG", "chB"]

        def nms_cat(catmask, a1, sh1, a2, sh2, first, ci):
            for t in range(NB):
                a = W * t
                lo = max(0, -sh1, -sh2)
                hi = min(W, W - sh1, W - sh2)
                tg = scrtags[(ci * NB + t) % 3]
                eng = nc.vector
                scr = pool.tile([128, W], dt.float32, tag=tg, name="scr")
                eng.tensor_tensor(scr[:, lo:hi],
                                  a1[:, a + lo + sh1:a + hi + sh1],
                                  a2[:, a + lo + sh2:a + hi + sh2], op.max)
                eng.tensor_tensor(scr[:, lo:hi], M2[:, a + lo:a + hi],
                                  scr[:, lo:hi], op.is_ge)
                if first:
                    nc.vector.tensor_copy(km[:, a + lo:a + hi], scr[:, lo:hi])
                    if lo > 0:
                        nc.vector.memset(km[:, a:a + lo], 0.0)
                        nc.vector.memset(km[:, a + hi:a + W], 0.0)
                else:
                    nc.vector.copy_predicated(km[:, a + lo:a + hi],
                                              catmask[:, a + lo:a + hi],
                                              scr[:, lo:hi])

        nms_cat(None, M2U, -1, M2D, 1, True, 0)     # cat3
        nms_cat(c2, M2U, 0, M2D, 0, False, 1)       # cat2
        nms_cat(c1, M2U, 1, M2D, -1, False, 2)      # cat1
        nms_cat(c0, M2, -1, M2, 1, False, 3)        # cat0
        if debug:
            nc.sync.dma_start(dbg["kmask"][:], km[:])

        # ---------------- thresholds -> f32 0/1 ------------------------------
        lowf = GX            # prod dead
        e0f = M2D            # m2d dead after cat1
        _stt(nc.vector, lowf[:], M2[:], float(C['KLOW']), km[:], op.is_gt, op.mult)
        _stt(nc.vector, e0f[:], M2[:], float(C['KHIGH']), km[:], op.is_gt, op.mult)
        if debug:
            nc.sync.dma_start(dbg["low0"][:], lowf[:])
            nc.sync.dma_start(dbg["e0"][:], e0f[:])

        # ---------------- pack to bits ---------------------------------------
        pia = pool.tile([128, FF], dt.int32, tag="Ct", name="pia")
        pib = pool.tile([128, FF], dt.int32, tag="Et", name="pib")

        def pack(dstp, srcf):
            nc.vector.tensor_copy(pia[:], srcf[:])          # f32 0/1 -> int32
            cur = pia[:]
            n = FF
            for lvl in range(5):
                n //= 2
                sh = 1 << lvl
                vw = cur.rearrange("p (n two) -> p n two", two=2)
                if lvl == 4:
                    dst = dstp[:]
                elif lvl % 2 == 0:
                    dst = pib[:, 0:n]
                else:
                    dst = pia[:, 0:n]
                _stt(nc.vector, dst, vw[:, :, 1], sh, vw[:, :, 0],
                     op.logical_shift_left, op.bitwise_or)
                cur = dst

        lowp = pool.tile([128, PF], dt.int32, tag="lp2", name="lowp")
        e0p = pool.tile([128, PF], dt.int32, tag="ep2", name="e0p")
        pack(lowp, lowf)
        pack(e0p, e0f)
        nc.vector.tensor_tensor(lowp[:], lowp[:], ipm[:], op.bitwise_and)
        nc.vector.tensor_tensor(e0p[:], e0p[:], ipm[:], op.bitwise_and)
        if debug:
            nc.sync.dma_start(dbg["lowp"][:], lowp[:])
            nc.sync.dma_start(dbg["e0p"][:], e0p[:])

        # ---------------- hysteresis flood fill ------------------------------
        e = e0p
        aa = pool.tile([128, PF], dt.int32, tag="haa", name="haa")
        bb2 = pool.tile([128, PF], dt.int32, tag="hbb", name="hbb")
        cc = pool.tile([128, PF], dt.int32, tag="hcc", name="hcc")
        dup = pool.tile([128, PF], dt.int32, tag="hdup", name="hdup")
        ddn = pool.tile([128, PF], dt.int32, tag="hddn", name="hddn")
        av = aa.rearrange("p (n w) -> p n w", w=PW)
        bv = bb2.rearrange("p (n w) -> p n w", w=PW)
        cv = cc.rearrange("p (n w) -> p n w", w=PW)
        mask_up = [min(i + 1, 31) for i in range(32)]   # dup[p] = c[p+1]
        mask_dn = [max(i - 1, 0) for i in range(32)]    # ddn[p] = c[p-1]
        for it in range(N_ITER):
            # horizontal dilate with cross-word carries
            _stt(nc.vector, aa[:], e[:], 1, e[:], op.logical_shift_left, op.bitwise_or)
            _stt(nc.vector, aa[:], e[:], 1, aa[:], op.logical_shift_right, op.bitwise_or)
            _stt(nc.vector, bb2[:, 1:PF], e[:, 0:PF - 1], 31, aa[:, 1:PF],
                 op.logical_shift_right, op.bitwise_or)
            nc.vector.tensor_copy(bb2[:, 0:1], aa[:, 0:1])
            nc.vector.tensor_copy(bv[:, :, 0], av[:, :, 0])
            _stt(nc.vector, cc[:, 0:PF - 1], e[:, 1:PF], 31, bb2[:, 0:PF - 1],
                 op.logical_shift_left, op.bitwise_or)
            nc.vector.tensor_copy(cc[:, PF - 1:PF], bb2[:, PF - 1:PF])
            nc.vector.tensor_copy(cv[:, :, PW - 1], bv[:, :, PW - 1])
            # vertical neighbors via stream_shuffle + boundary DMAs
            nc.vector.stream_shuffle(dup[:], cc[:], mask_up)
            nc.vector.stream_shuffle(ddn[:], cc[:], mask_dn)
            q1 = nc.sync if it % 2 == 0 else nc.scalar
            q2 = nc.scalar if it % 2 == 0 else nc.sync
            # rows masked to zero by lowp's interior mask need no zero fill
            q1.dma_start(dup[31:127:32, :], cc[32:128:32, :])
            q1.dma_start(dup[127:128, 0:PF - PW], cc[0:1, PW:PF])
            q2.dma_start(ddn[32:128:32, :], cc[31:127:32, :])
            q2.dma_start(ddn[0:1, PW:PF], cc[127:128, 0:PF - PW])
            # e' = (dup | ddn | c) & low
            nc.vector.tensor_tensor(dup[:], dup[:], ddn[:], op.bitwise_or)
            nc.vector.tensor_tensor(dup[:], dup[:], cc[:], op.bitwise_or)
            nc.vector.tensor_tensor(e[:], dup[:], lowp[:], op.bitwise_and)
        if debug:
            nc.sync.dma_start(dbg["ep"][:], e[:])

        # ---------------- unpack complement -> output ------------------------
        ne = aa
        _stt(nc.vector, ne[:], e[:], -1, e[:], op.bitwise_xor, op.bypass)
        outf = f32buf("A", "outf")
        cur = ne[:]
        n = PF
        masks = [0xFFFF, 0xFF, 0xF, 0x3, 0x1]
        for lvl in range(5):
            half = 32 >> (lvl + 1)       # bits per half-word after split
            m = masks[lvl]
            dst = (pib[:, 0:2 * n] if lvl % 2 == 0 else pia[:, 0:2 * n])
            dv2 = dst.rearrange("p (n two) -> p n two", two=2)
            _ts_int(nc.vector, dv2[:, :, 0], cur, m, op.bitwise_and)
            _ts_int(nc.vector, dv2[:, :, 1], cur, half, op.logical_shift_right,
                    m, op.bitwise_and)
            cur = dst
            n *= 2
        nc.vector.tensor_copy(outf[:], pib[:])    # int 0/1 -> f32
        for t in range(NB):
            q = nc.sync if t % 2 == 0 else nc.scalar
            q.dma_start(out_d[128 * t:128 * (t + 1), :], outf[:, W * t:W * (t + 1)])

    nc.compile()
    return nc, C, dbg


def _run(inputs, debug=False, trace=False):
    from concourse.bass_utils import run_bass_kernel_spmd
    key = ("dbg" if debug else "plain")
    if key not in _cache:
        _cache[key] = build_program(debug=debug)
    nc, C, dbg = _cache[key]
    x = np.asarray(inputs["x"], dtype=np.float32)
    in_maps = []
    for c in range(B):
        in_maps.append({
            "x": np.ascontiguousarray(x[c]),
            "bandT": C['bandT'],
            "band2T": C['band2T'],
            "hcC": C['hcC'],
            "interior": C['interior_packed'],
            "zeros_f32": C['zeros_f32'],
            "zeros_i32": C['zeros_i32'],
        })
    res = run_bass_kernel_spmd(nc, in_maps, core_ids=list(range(B)), trace=trace)
    return res


def kernel(x, gaussian_kernel=None, sobel_x=None, sobel_y=None):
    res = _run({"x": x})
    out = np.stack([res.results[c]["out"] for c in range(B)], axis=0)
    return out.reshape(B, 1, H, W).astype(np.float32)


# revision 21
# speedup vs baseline: 1.1207x; 1.0549x over previous
"""Canny edge detector on 8 Trainium2 NeuronCores — pure data parallel,
one 1024x1024 image per core.

Per-core pipeline (all decisions in f32, no sqrt/atan2 anywhere):
  gray -> vertical gaussian^2 (PE banded f32 matmuls) -> horizontal
  gaussian (2x 5-tap fused STT passes) -> sobel vertical parts as PE banded
  matmuls, horizontal parts as free-dim taps -> squared-magnitude NMS with
  copy_predicated direction select (row-shifted squared magnitudes via PE
  shift matmuls) -> thresholds on squared magnitude -> 32x bit-packed
  hysteresis flood fill (fused shift/or int passes + stream_shuffle row
  shifts with small DMA boundary fixes).
"""
import math
import numpy as np

B, H, W = 8, 1024, 1024
NB = H // 128          # 8 row blocks
PW = W // 32           # 32 packed words per row per block
PF = NB * PW           # 256 packed words per partition
FF = NB * W            # 8192 f32 elems per partition
N_ITER = 17

_cache = {}


# ---------------------------------------------------------------- constants
def _gauss_v():
    x = np.linspace(-2, 2, 5).astype(np.float64)
    g2 = np.exp(-(x.reshape(5, 1) ** 2 + x.reshape(1, 5) ** 2) / 2.0)
    K = g2 / g2.sum()
    v = K[:, 2] / math.sqrt(K[2, 2])
    return v  # 5-tap 1D gaussian, outer(v,v) = 2D kernel


def _band_matrix(n, taps):
    M = np.zeros((n, n), dtype=np.float64)
    for d, w in taps.items():
        i = np.arange(n)
        j = i + d
        m = (j >= 0) & (j < n)
        M[i[m], j[m]] = w
    return M


def _threshold_sq(t):
    import struct
    t = np.float32(t)

    def f2i(f):
        return struct.unpack('<I', struct.pack('<f', np.float32(f)))[0]

    def i2f(i):
        return np.float32(struct.unpack('<f', struct.pack('<I', i))[0])

    lo_i = f2i(np.float32(0.0))
    hi_i = f2i(np.float32(float(t) * float(t) * 4.0))
    while lo_i + 1 < hi_i:
        mid = (lo_i + hi_i) // 2
        if np.sqrt(i2f(mid), dtype=np.float32) <= t:
            lo_i = mid
        else:
            hi_i = mid
    return i2f(lo_i)


def _build_consts():
    v = _gauss_v()
    Bm = _band_matrix(H, {d - 2: v[d] for d in range(5)})
    BV2 = (Bm @ Bm).astype(np.float32)          # vertical gaussian applied twice
    blocks = []          # list of (t, s)
    mats = []
    for t in range(NB):
        for s in (t - 1, t, t + 1):
            if 0 <= s < NB:
                blk = BV2[128 * t:128 * (t + 1), 128 * s:128 * (s + 1)]
                blocks.append((t, s))
                mats.append(np.ascontiguousarray(blk.T))
    # sobel vertical operators as 128x128 blocks (same for every t)
    sv = np.zeros((128, 128), np.float32)        # s1[p] = g[p-1] + 2 g[p] + g[p+1]
    dv = np.zeros((128, 128), np.float32)        # d1[p] = g[p+1] - g[p-1]
    for p in range(128):
        sv[p, p] = 2.0
        if p > 0:
            sv[p, p - 1] = 1.0
            dv[p, p - 1] = -1.0
        if p < 127:
            sv[p, p + 1] = 1.0
            dv[p, p + 1] = 1.0
    svu = np.zeros((128, 128), np.float32); svu[0, 127] = 1.0     # from block t-1
    svd = np.zeros((128, 128), np.float32); svd[127, 0] = 1.0     # from block t+1
    dvu = np.zeros((128, 128), np.float32); dvu[0, 127] = -1.0
    dvd = np.zeros((128, 128), np.float32); dvd[127, 0] = 1.0
    shup = np.zeros((128, 128), np.float32)      # u[p] = x[p-1]
    shdn = np.zeros((128, 128), np.float32)      # d[p] = x[p+1]
    for p in range(128):
        if p > 0:
            shup[p, p - 1] = 1.0
        if p < 127:
            shdn[p, p + 1] = 1.0
    ident = np.eye(128, dtype=np.float32)
    extra_names = ['sv', 'svu', 'svd', 'dv', 'dvu', 'dvd', 'shup', 'shdn', 'ident']
    extra_idx = {}
    mats2 = []
    for nm, M in zip(extra_names, [sv, svu, svd, dv, dvu, dvd, shup, shdn, ident]):
        extra_idx[nm] = len(mats2)
        mats2.append(np.ascontiguousarray(M.T))
    bandT = np.ascontiguousarray(np.concatenate(mats, axis=1).astype(np.float32))
    band2T = np.ascontiguousarray(np.concatenate(mats2, axis=1).astype(np.float32))

    # horizontal gaussian^2 as matrix product C = Bw @ Bw (exact border rows/cols),
    # stored as a Toeplitz master strip + exact first/last block columns
    C64 = Bm @ Bm
    Cf = C64.astype(np.float32)
    w9 = np.array([C64[512, 512 + k - 4] for k in range(9)])
    masterS = np.zeros((128, 1152), np.float64)
    for p in range(128):
        lo = max(0, 512 + p - 4)
        for j in range(lo, min(1152, 512 + p + 5)):
            masterS[p, j] = w9[j - 512 - p + 4]
    masterS = masterS.astype(np.float32)
    for n in range(2):
        for vv in ([0, 1, 2, 3, 4] if n == 0 else [3, 4, 5, 6, 7]):
            if (vv, n) in [(0, 0), (7, 1)]:
                continue
            o = 512 + 512 * n - 128 * vv
            assert (Cf[128 * vv:128 * vv + 128, 512 * n:512 * n + 512]
                    == masterS[:, o:o + 512]).all()
    hcC = np.concatenate([masterS, Cf[0:128, 0:512], Cf[896:1024, 512:1024]],
                         axis=1)
    hcC = np.ascontiguousarray(hcC.astype(np.float32))

    # interior mask, packed: bit b of word (p, t*PW + j) is col 32j+b of row 128t+p
    interior = np.zeros((H, W), np.uint32)
    interior[1:-1, 1:-1] = 1
    ip = np.zeros((128, PF), np.uint32)
    for t in range(NB):
        rows = interior[128 * t:128 * (t + 1)]          # [128, W]
        bits = rows.reshape(128, PW, 32)
        words = (bits << np.arange(32, dtype=np.uint32)).sum(axis=2, dtype=np.uint32)
        ip[:, t * PW:(t + 1) * PW] = words
    ip = ip.view(np.int32)

    taps = [np.float32(x) for x in v]      # 5-tap horizontal gaussian
    consts = dict(
        bandT=bandT, band2T=band2T, hcC=hcC, blocks=blocks, extra_idx=extra_idx,
        interior_packed=np.ascontiguousarray(ip),
        taps=taps,
        KLOW=_threshold_sq(0.1), KHIGH=_threshold_sq(0.2),
        T1SQ=np.float32(np.tan(np.pi / 8) ** 2),
        T2SQ=np.float32(np.tan(3 * np.pi / 8) ** 2),
        zeros_f32=np.zeros((1, W), np.float32),
        zeros_i32=np.zeros((1, PW), np.int32),
    )
    return consts


# ---------------------------------------------------------------- helpers
def _stt(eng, out, in0, scalar, in1, op0, op1):
    from concourse import mybir as mb
    if isinstance(scalar, (int, np.integer)) and not isinstance(scalar, bool):
        imm = mb.ImmediateValue(dtype=mb.dt.int32, value=int(scalar))
    else:
        imm = mb.ImmediateValue(dtype=mb.dt.float32, value=float(scalar))
    return eng.add_instruction(
        mb.InstTensorScalarPtr(
            name=eng.bass.get_next_instruction_name(),
            is_scalar_tensor_tensor=True,
            op0=op0, op1=op1,
            ins=[eng.lower_ap(in0), imm, eng.lower_ap(in1)],
            outs=[eng.lower_ap(out)],
        ))


def _ts_int(eng, out, in0, s0, op0, s1=None, op1=None):
    from concourse import mybir as mb
    ins = [eng.lower_ap(in0), mb.ImmediateValue(dtype=mb.dt.int32, value=int(s0))]
    kw = dict(op0=op0)
    if s1 is not None:
        ins.append(mb.ImmediateValue(dtype=mb.dt.int32, value=int(s1)))
        kw['op1'] = op1
    return eng.add_instruction(
        mb.InstTensorScalarPtr(
            name=eng.bass.get_next_instruction_name(),
            ins=ins,
            outs=[eng.lower_ap(out)],
            **kw,
        ))


# ---------------------------------------------------------------- program
def build_program(debug=False):
    import concourse.tile as tile
    from concourse import bacc, mybir
    from contextlib import ExitStack
    dt = mybir.dt
    op = mybir.AluOpType
    AF = mybir.ActivationFunctionType
    C = _build_consts()
    EI = C['extra_idx']

    nc = bacc.Bacc("TRN2", target_bir_lowering=False, debug=False)
    nblk = C['bandT'].shape[1] // 128
    nblk2 = C['band2T'].shape[1] // 128
    x_d = nc.dram_tensor("x", [3, H, W], dt.float32, kind="ExternalInput").ap()
    band_d = nc.dram_tensor("bandT", [128, nblk * 128], dt.float32, kind="ExternalInput").ap()
    band2_d = nc.dram_tensor("band2T", [128, nblk2 * 128], dt.float32, kind="ExternalInput").ap()
    hc_d = nc.dram_tensor("hcC", [128, 2176], dt.float32, kind="ExternalInput").ap()
    ip_d = nc.dram_tensor("interior", [128, PF], dt.int32, kind="ExternalInput").ap()
    zf_d = nc.dram_tensor("zeros_f32", [1, W], dt.float32, kind="ExternalInput").ap()
    zi_d = nc.dram_tensor("zeros_i32", [1, PW], dt.int32, kind="ExternalInput").ap()
    out_d = nc.dram_tensor("out", [H, W], dt.float32, kind="ExternalOutput").ap()
    dbg = {}
    if debug:
        for name in ("gray", "g1", "gfull", "gx", "gy", "m2", "kmask", "low0", "e0"):
            dbg[name] = nc.dram_tensor("dbg_" + name, [128, FF], dt.float32,
                                       kind="ExternalOutput").ap()
        for name in ("lowp", "e0p", "ep"):
            dbg[name] = nc.dram_tensor("dbg_" + name, [128, PF], dt.int32,
                                       kind="ExternalOutput").ap()

    with tile.TileContext(nc) as tc, ExitStack() as ctx:
        pool = ctx.enter_context(tc.tile_pool(name="main", bufs=1))
        psA = ctx.enter_context(tc.tile_pool(name="psA", bufs=1, space="PSUM"))
        psB = ctx.enter_context(tc.tile_pool(name="psB", bufs=1, space="PSUM"))

        def f32buf(tag, name):
            return pool.tile([128, FF], dt.float32, tag=tag, name=name)

        def blk(buf, t, n=None):
            if n is None:
                return buf[:, W * t:W * (t + 1)]
            return buf[:, W * t + 512 * n: W * t + 512 * (n + 1)]

        band = pool.tile([128, nblk * 128], dt.float32, tag="Ct", name="band")
        nc.sync.dma_start(band[:], band_d[:])
        band2 = pool.tile([128, nblk2 * 128], dt.float32, tag="band2", name="band2")
        nc.scalar.dma_start(band2[:], band2_d[:])
        ipm = pool.tile([128, PF], dt.int32, tag="ipm", name="ipm")
        nc.scalar.dma_start(ipm[:], ip_d[:])

        def bmat(i):
            return band[:, 128 * i:128 * (i + 1)]

        def bmat2(i):
            return band2[:, 128 * i:128 * (i + 1)]

        # dummy matmul to absorb the const-DMA semaphore on PE early
        dps = psB.tile([128, 128], dt.float32, tag="mmD1", name="dummy", bufs=2)
        nc.tensor.matmul(dps[:], bmat(0), bmat(0), start=True, stop=True)

        # ---------------- gray ---------------------------------------------
        A = f32buf("A", "gray")
        for t in range(NB):
            r = pool.tile([128, W], dt.float32, tag="chR", name="chR")
            g = pool.tile([128, W], dt.float32, tag="chG", name="chG")
            b = pool.tile([128, W], dt.float32, tag="chB", name="chB")
            nc.sync.dma_start(r[:], x_d[0, 128 * t:128 * (t + 1), :])
            nc.scalar.dma_start(g[:], x_d[1, 128 * t:128 * (t + 1), :])
            nc.sync.dma_start(b[:], x_d[2, 128 * t:128 * (t + 1), :])
            sl = blk(A, t)
            nc.scalar.activation(sl, r[:], AF.Copy, scale=0.299)
            _stt(nc.vector, sl, g[:], 0.587, sl, op.mult, op.add)
            _stt(nc.vector, sl, b[:], 0.114, sl, op.mult, op.add)
        if debug:
            nc.sync.dma_start(dbg["gray"][:], A[:])

        # ---------------- vertical gaussian^2 on PE -> Bt -------------------
        Bb = f32buf("Bt", "g1")
        bmap = {}
        for i, (t, s) in enumerate(C['blocks']):
            bmap.setdefault(t, []).append((s, i))
        for t in range(NB):
            for n in range(2):
                ps = psA.tile([128, 512], dt.float32, tag="mmB", name="mmB", bufs=2)
                lst = bmap[t]
                for j, (s, i) in enumerate(lst):
                    nc.tensor.matmul(ps[:], bmat(i), blk(A, s, n),
                                     start=(j == 0), stop=(j == len(lst) - 1))
                nc.scalar.activation(blk(Bb, t, n), ps[:], AF.Copy)
        if debug:
            nc.sync.dma_start(dbg["g1"][:], Bb[:])

        # ---------------- horizontal gaussian (5-tap, twice) ----------------
        taps = C['taps']

        def hconv5(dst, src):
            # out[j] = sum_d v[d] src[j+d-2], zero padded, per 1024-block
            for t in range(NB):
                a = W * t
                ve = nc.vector
                nc.scalar.activation(dst[:, a:a + W], src[:, a:a + W], AF.Copy,
                                     scale=float(taps[2]))
                _stt(ve, dst[:, a + 2:a + W], src[:, a:a + W - 2], float(taps[0]),
                     dst[:, a + 2:a + W], op.mult, op.add)
                _stt(ve, dst[:, a + 1:a + W], src[:, a:a + W - 1], float(taps[1]),
                     dst[:, a + 1:a + W], op.mult, op.add)
                _stt(ve, dst[:, a:a + W - 1], src[:, a + 1:a + W], float(taps[3]),
                     dst[:, a:a + W - 1], op.mult, op.add)
                _stt(ve, dst[:, a:a + W - 2], src[:, a + 2:a + W], float(taps[4]),
                     dst[:, a:a + W - 2], op.mult, op.add)

        # g_full = g1 @ C on PE: per row-tile, transpose 128x128 subtiles of g1
        # (PE transpose mode), then banded matmuls against the C master strip.
        hcC = pool.tile([128, 2176], dt.float32, tag="Dt", name="hcC")
        nc.sync.dma_start(hcC[:], hc_d[:])
        GF = A              # gray dead; g_full lands in A
        def _hp(db, doff, sb, soff):
            # one 5-tap pass on a 1024 block: db[doff:] = BW(sb[soff:])
            nc.scalar.activation(db[:, doff:doff + W], sb[:, soff:soff + W],
                                 AF.Copy, scale=float(taps[2]))
            _stt(nc.vector, db[:, doff + 2:doff + W], sb[:, soff:soff + W - 2],
                 float(taps[0]), db[:, doff + 2:doff + W], op.mult, op.add)
            _stt(nc.vector, db[:, doff + 1:doff + W], sb[:, soff:soff + W - 1],
                 float(taps[1]), db[:, doff + 1:doff + W], op.mult, op.add)
            _stt(nc.vector, db[:, doff:doff + W - 1], sb[:, soff + 1:soff + W],
                 float(taps[3]), db[:, doff:doff + W - 1], op.mult, op.add)
            _stt(nc.vector, db[:, doff:doff + W - 2], sb[:, soff + 2:soff + W],
                 float(taps[4]), db[:, doff:doff + W - 2], op.mult, op.add)

        for t in range(4, NB):     # DVE path for blocks 4..7 (overlaps PE blocks)
            a = W * t
            tmp = pool.tile([128, W], dt.float32, tag="chG", name="htmp")
            _hp(tmp, 0, Bb, a)
            _hp(GF, a, tmp, 0)
        for t in range(4):
            g1T = pool.tile([128, 1024], dt.float32, tag="chR", name="g1T")
            for k in range(8):
                pst = psB.tile([128, 128], dt.float32, tag="mmD1", name="tp", bufs=2)
                nc.tensor.transpose(pst[:], Bb[:, W * t + 128 * k: W * t + 128 * (k + 1)],
                                    bmat2(EI['ident']))
                nc.scalar.activation(g1T[:, 128 * k:128 * (k + 1)], pst[:], AF.Copy)
            for n in range(2):
                pso = psA.tile([128, 512], dt.float32, tag="mmB", name="hco", bufs=2)
                vs = [0, 1, 2, 3, 4] if n == 0 else [3, 4, 5, 6, 7]
                for j, vv in enumerate(vs):
                    if (vv, n) == (0, 0):
                        rhs = hcC[:, 1152:1664]
                    elif (vv, n) == (7, 1):
                        rhs = hcC[:, 1664:2176]
                    else:
                        o = 512 + 512 * n - 128 * vv
                        rhs = hcC[:, o:o + 512]
                    nc.tensor.matmul(pso[:], g1T[:, 128 * vv:128 * (vv + 1)], rhs,
                                     start=(j == 0), stop=(j == len(vs) - 1))
                nc.scalar.activation(blk(GF, t, n), pso[:], AF.Copy)
        if debug:
            nc.sync.dma_start(dbg["gfull"][:], GF[:])

        # ---------------- sobel vertical parts: u/d row shifts on PE ---------
        # u[p] = g[p-1], d[p] = g[p+1]; s1 = u + 2g + d ; d1 = d - u (DVE)
        U = f32buf("Bt", "ush")      # g1 dead after hconv
        Dd = f32buf("Ct", "dsh")     # band1 dead
        for t in range(NB):
            for n in range(2):
                ps = psA.tile([128, 512], dt.float32, tag="mmS", name="mmS", bufs=2)
                nc.tensor.matmul(ps[:], bmat2(EI['shup']), blk(GF, t, n),
                                 start=True, stop=True)
                nc.scalar.activation(blk(U, t, n), ps[:], AF.Copy)
                ps2 = psB.tile([128, 512], dt.float32, tag="mmD1", name="mmD1", bufs=2)
                nc.tensor.matmul(ps2[:], bmat2(EI['shdn']), blk(GF, t, n),
                                 start=True, stop=True)
                nc.scalar.activation(blk(Dd, t, n), ps2[:], AF.Copy)
        # block-crossing boundary rows
        nc.sync.dma_start(U[0:1, W:FF], GF[127:128, 0:FF - W])
        nc.vector.memset(U[0:1, 0:W], 0.0)
        nc.scalar.dma_start(Dd[127:128, 0:FF - W], GF[0:1, W:FF])
        nc.scalar.dma_start(Dd[127:128, FF - W:FF], zf_d[:])
        D1 = f32buf("Dt", "d1")      # hcC dead
        nc.vector.tensor_tensor(D1[:], Dd[:], U[:], op.subtract)
        _stt(nc.vector, U[:], GF[:], 2.0, U[:], op.mult, op.add)
        nc.vector.tensor_tensor(U[:], U[:], Dd[:], op.add)
        S1 = U      # s1 now lives in Bt

        # gx = D_W(s1) -> Ct ; gy = S_W(d1) -> Et
        GX = Dd
        GY = f32buf("Et", "gy")
        for t in range(NB):
            a = W * t
            ve = nc.vector
            # gx[j] = s1[j+1] - s1[j-1]
            nc.scalar.activation(GX[:, a:a + W - 1], S1[:, a + 1:a + W], AF.Copy)
            nc.vector.memset(GX[:, a + W - 1:a + W], 0.0)
            _stt(ve, GX[:, a + 1:a + W], S1[:, a:a + W - 1], -1.0,
                 GX[:, a + 1:a + W], op.mult, op.add)
            # gy[j] = d1[j-1] + 2 d1[j] + d1[j+1]
            nc.scalar.activation(GY[:, a:a + W], D1[:, a:a + W], AF.Copy, scale=2.0)
            _stt(ve, GY[:, a + 1:a + W], D1[:, a:a + W - 1], 1.0,
                 GY[:, a + 1:a + W], op.mult, op.add)
            _stt(ve, GY[:, a:a + W - 1], D1[:, a + 1:a + W], 1.0,
                 GY[:, a:a + W - 1], op.mult, op.add)
        if debug:
            nc.sync.dma_start(dbg["gx"][:], GX[:])
            nc.sync.dma_start(dbg["gy"][:], GY[:])

        # ---------------- m2 / direction masks ------------------------------
        M2X = A             # g_full dead after sobel PE
        M2Y = D1            # d1 dead after gy
        nc.scalar.activation(M2X[:], GX[:], AF.Square)
        nc.scalar.activation(M2Y[:], GY[:], AF.Square)
        c0 = pool.tile([128, FF], dt.uint8, tag="haa", name="c0")
        c2 = pool.tile([128, FF], dt.uint8, tag="hcc", name="c2")
        c1 = pool.tile([128, FF], dt.uint8, tag="hbb", name="c1")
        _stt(nc.vector, c0[:], M2X[:], float(C['T1SQ']), M2Y[:], op.mult, op.is_ge)
        _stt(nc.vector, c2[:], M2X[:], float(C['T2SQ']), M2Y[:], op.mult, op.is_le)
        M2 = S1             # s1 dead after gx (Bt slot)
        nc.gpsimd.tensor_tensor(M2[:], M2X[:], M2Y[:], op.add)
        PROD = GX           # overwrite gx in place
        nc.vector.tensor_tensor(PROD[:], GX[:], GY[:], op.mult)
        # c1 = (prod >= 0) & !(c0 | c2)
        nc.vector.tensor_tensor(c1[:], c0[:], c2[:], op.logical_or)
        nc.vector.tensor_scalar(c1[:], c1[:], 0.0, None, op.is_equal)
        _stt(nc.vector, c1[:], PROD[:], 0.0, c1[:], op.is_ge, op.logical_and)
        if debug:
            nc.sync.dma_start(dbg["m2"][:], M2[:])

        # ---------------- NMS ------------------------------------------------
        # m2u = m2[row-1] -> M2X slot ; m2d = m2[row+1] -> M2Y slot (PE shifts)
        M2U = A
        M2D = D1
        for t in range(NB):
            for n in range(2):
                ps = psA.tile([128, 512], dt.float32, tag="mmU", name="mmU", bufs=1)
                nc.tensor.matmul(ps[:], bmat2(EI['shup']), blk(M2, t, n),
                                 start=True, stop=True)
                nc.scalar.activation(blk(M2U, t, n), ps[:], AF.Copy)
                ps2 = psB.tile([128, 512], dt.float32, tag="mmV", name="mmV", bufs=1)
                nc.tensor.matmul(ps2[:], bmat2(EI['shdn']), blk(M2, t, n),
                                 start=True, stop=True)
                nc.scalar.activation(blk(M2D, t, n), ps2[:], AF.Copy)
        # boundary rows across blocks
        nc.sync.dma_start(M2U[0:1, W:FF], M2[127:128, 0:FF - W])
        nc.vector.memset(M2U[0:1, 0:W], 0.0)
        nc.scalar.dma_start(M2D[127:128, 0:FF - W], M2[0:1, W:FF])
        nc.scalar.dma_start(M2D[127:128, FF - W:FF], zf_d[:])

        km = GY             # gy dead after prod
        scrtags = ["chR", "chG", "chB"]

        def nms_cat(catmask, a1, sh1, a2, sh2, first, ci):
            for t in range(NB):
                a = W * t
                lo = max(0, -sh1, -sh2)
                hi = min(W, W - sh1, W - sh2)
                tg = scrtags[(ci * NB + t) % 3]
                eng = nc.vector
                scr = pool.tile([128, W], dt.float32, tag=tg, name="scr")
                eng.tensor_tensor(scr[:, lo:hi],
                                  a1[:, a + lo + sh1:a + hi + sh1],
                                  a2[:, a + lo + sh2:a + hi + sh2], op.max)
                eng.tensor_tensor(scr[:, lo:hi], M2[:, a + lo:a + hi],
                                  scr[:, lo:hi], op.is_ge)
                if first:
                    nc.vector.tensor_copy(km[:, a + lo:a + hi], scr[:, lo:hi])
                    if lo > 0:
                        nc.vector.memset(km[:, a:a + lo], 0.0)
                        nc.vector.memset(km[:, a + hi:a + W], 0.0)
                else:
                    nc.vector.copy_predicated(km[:, a + lo:a + hi],
                                              catmask[:, a + lo:a + hi],
                                              scr[:, lo:hi])

        nms_cat(None, M2U, -1, M2D, 1, True, 0)     # cat3
        nms_cat(c2, M2U, 0, M2D, 0, False, 1)       # cat2
        nms_cat(c1, M2U, 1, M2D, -1, False, 2)      # cat1
        nms_cat(c0, M2, -1, M2, 1, False, 3)        # cat0
        if debug:
            nc.sync.dma_start(dbg["kmask"][:], km[:])

        # ---------------- thresholds -> f32 0/1 ------------------------------
        lowf = GX            # prod dead
        e0f = M2D            # m2d dead after cat1
        _stt(nc.vector, lowf[:], M2[:], float(C['KLOW']), km[:], op.is_gt, op.mult)
        _stt(nc.vector, e0f[:], M2[:], float(C['KHIGH']), km[:], op.is_gt, op.mult)
        if debug:
            nc.sync.dma_start(dbg["low0"][:], lowf[:])
            nc.sync.dma_start(dbg["e0"][:], e0f[:])

        # ---------------- pack to bits ---------------------------------------
        pia = pool.tile([128, FF], dt.int32, tag="A", name="pia")
        pib = pool.tile([128, FF], dt.int32, tag="Et", name="pib")

        def pack(dstp, srcf):
            nc.vector.tensor_copy(pia[:], srcf[:])          # f32 0/1 -> int32
            cur = pia[:]
            n = FF
            for lvl in range(5):
                n //= 2
                sh = 1 << lvl
                vw = cur.rearrange("p (n two) -> p n two", two=2)
                if lvl == 4:
                    dst = dstp[:]
                elif lvl % 2 == 0:
                    dst = pib[:, 0:n]
                else:
                    dst = pia[:, 0:n]
                _stt(nc.vector, dst, vw[:, :, 1], sh, vw[:, :, 0],
                     op.logical_shift_left, op.bitwise_or)
                cur = dst

        lowp = pool.tile([128, PF], dt.int32, tag="lp2", name="lowp")
        e0p = pool.tile([128, PF], dt.int32, tag="ep2", name="e0p")
        pack(lowp, lowf)
        pack(e0p, e0f)
        nc.vector.tensor_tensor(lowp[:], lowp[:], ipm[:], op.bitwise_and)
        nc.vector.tensor_tensor(e0p[:], e0p[:], ipm[:], op.bitwise_and)
        if debug:
            nc.sync.dma_start(dbg["lowp"][:], lowp[:])
            nc.sync.dma_start(dbg["e0p"][:], e0p[:])

        # ---------------- hysteresis flood fill ------------------------------
        e = e0p
        aa = pool.tile([128, PF], dt.int32, tag="haa", name="haa")
        bb2 = pool.tile([128, PF], dt.int32, tag="hbb", name="hbb")
        cc = pool.tile([128, PF], dt.int32, tag="hcc", name="hcc")
        dup = pool.tile([128, PF], dt.int32, tag="hdup", name="hdup")
        ddn = pool.tile([128, PF], dt.int32, tag="hddn", name="hddn")
        av = aa.rearrange("p (n w) -> p n w", w=PW)
        bv = bb2.rearrange("p (n w) -> p n w", w=PW)
        cv = cc.rearrange("p (n w) -> p n w", w=PW)
        mask_up = [min(i + 1, 31) for i in range(32)]   # dup[p] = c[p+1]
        mask_dn = [max(i - 1, 0) for i in range(32)]    # ddn[p] = c[p-1]
        for it in range(N_ITER):
            # horizontal dilate with cross-word carries
            _stt(nc.vector, aa[:], e[:], 1, e[:], op.logical_shift_left, op.bitwise_or)
            _stt(nc.vector, aa[:], e[:], 1, aa[:], op.logical_shift_right, op.bitwise_or)
            _stt(nc.vector, bb2[:, 1:PF], e[:, 0:PF - 1], 31, aa[:, 1:PF],
                 op.logical_shift_right, op.bitwise_or)
            nc.vector.tensor_copy(bb2[:, 0:1], aa[:, 0:1])
            nc.vector.tensor_copy(bv[:, :, 0], av[:, :, 0])
            _stt(nc.vector, cc[:, 0:PF - 1], e[:, 1:PF], 31, bb2[:, 0:PF - 1],
                 op.logical_shift_left, op.bitwise_or)
            nc.vector.tensor_copy(cc[:, PF - 1:PF], bb2[:, PF - 1:PF])
            nc.vector.tensor_copy(cv[:, :, PW - 1], bv[:, :, PW - 1])
            # vertical neighbors via stream_shuffle + boundary DMAs
            nc.vector.stream_shuffle(dup[:], cc[:], mask_up)
            nc.vector.stream_shuffle(ddn[:], cc[:], mask_dn)
            q1 = nc.sync if it % 2 == 0 else nc.scalar
            q2 = nc.scalar if it % 2 == 0 else nc.sync
            # rows masked to zero by lowp's interior mask need no zero fill
            q1.dma_start(dup[31:127:32, :], cc[32:128:32, :])
            q1.dma_start(dup[127:128, 0:PF - PW], cc[0:1, PW:PF])
            q2.dma_start(ddn[32:128:32, :], cc[31:127:32, :])
            q2.dma_start(ddn[0:1, PW:PF], cc[127:128, 0:PF - PW])
            # e' = (dup | ddn | c) & low
            nc.vector.tensor_tensor(dup[:], dup[:], ddn[:], op.bitwise_or)
            nc.vector.tensor_tensor(dup[:], dup[:], cc[:], op.bitwise_or)
            nc.vector.tensor_tensor(e[:], dup[:], lowp[:], op.bitwise_and)
        if debug:
            nc.sync.dma_start(dbg["ep"][:], e[:])

        # ---------------- unpack complement -> output ------------------------
        ne = aa
        _stt(nc.vector, ne[:], e[:], -1, e[:], op.bitwise_xor, op.bypass)
        outf = f32buf("A", "outf")
        cur = ne[:]
        n = PF
        masks = [0xFFFF, 0xFF, 0xF, 0x3, 0x1]
        for lvl in range(5):
            half = 32 >> (lvl + 1)       # bits per half-word after split
            m = masks[lvl]
            dst = (pib[:, 0:2 * n] if lvl % 2 == 0 else pia[:, 0:2 * n])
            dv2 = dst.rearrange("p (n two) -> p n two", two=2)
            _ts_int(nc.vector, dv2[:, :, 0], cur, m, op.bitwise_and)
            _ts_int(nc.vector, dv2[:, :, 1], cur, half, op.logical_shift_right,
                    m, op.bitwise_and)
            cur = dst
            n *= 2
        nc.vector.tensor_copy(outf[:], pib[:])    # int 0/1 -> f32
        for t in range(NB):
            q = nc.sync if t % 2 == 0 else nc.scalar
            q.dma_start(out_d[128 * t:128 * (t + 1), :], outf[:, W * t:W * (t + 1)])

    nc.compile()
    return nc, C, dbg


def _run(inputs, debug=False, trace=False):
    from concourse.bass_utils import run_bass_kernel_spmd
    key = ("dbg" if debug else "plain")
    if key not in _cache:
        _cache[key] = build_program(debug=debug)
    nc, C, dbg = _cache[key]
    x = np.asarray(inputs["x"], dtype=np.float32)
    in_maps = []
    for c in range(B):
        in_maps.append({
            "x": np.ascontiguousarray(x[c]),
            "bandT": C['bandT'],
            "band2T": C['band2T'],
            "hcC": C['hcC'],
            "interior": C['interior_packed'],
            "zeros_f32": C['zeros_f32'],
            "zeros_i32": C['zeros_i32'],
        })
    res = run_bass_kernel_spmd(nc, in_maps, core_ids=list(range(B)), trace=trace)
    return res


def kernel(x, gaussian_kernel=None, sobel_x=None, sobel_y=None):
    res = _run({"x": x})
    out = np.stack([res.results[c]["out"] for c in range(B)], axis=0)
    return out.reshape(B, 1, H, W).astype(np.float32)


# revision 22
# speedup vs baseline: 1.1253x; 1.0041x over previous
"""Canny edge detector on 8 Trainium2 NeuronCores — pure data parallel,
one 1024x1024 image per core.

Per-core pipeline (all decisions in f32, no sqrt/atan2 anywhere):
  gray -> vertical gaussian^2 (PE banded f32 matmuls) -> horizontal
  gaussian (2x 5-tap fused STT passes) -> sobel vertical parts as PE banded
  matmuls, horizontal parts as free-dim taps -> squared-magnitude NMS with
  copy_predicated direction select (row-shifted squared magnitudes via PE
  shift matmuls) -> thresholds on squared magnitude -> 32x bit-packed
  hysteresis flood fill (fused shift/or int passes + stream_shuffle row
  shifts with small DMA boundary fixes).
"""
import math
import numpy as np

B, H, W = 8, 1024, 1024
NB = H // 128          # 8 row blocks
PW = W // 32           # 32 packed words per row per block
PF = NB * PW           # 256 packed words per partition
FF = NB * W            # 8192 f32 elems per partition
N_ITER = 17

_cache = {}


# ---------------------------------------------------------------- constants
def _gauss_v():
    x = np.linspace(-2, 2, 5).astype(np.float64)
    g2 = np.exp(-(x.reshape(5, 1) ** 2 + x.reshape(1, 5) ** 2) / 2.0)
    K = g2 / g2.sum()
    v = K[:, 2] / math.sqrt(K[2, 2])
    return v  # 5-tap 1D gaussian, outer(v,v) = 2D kernel


def _band_matrix(n, taps):
    M = np.zeros((n, n), dtype=np.float64)
    for d, w in taps.items():
        i = np.arange(n)
        j = i + d
        m = (j >= 0) & (j < n)
        M[i[m], j[m]] = w
    return M


def _threshold_sq(t):
    import struct
    t = np.float32(t)

    def f2i(f):
        return struct.unpack('<I', struct.pack('<f', np.float32(f)))[0]

    def i2f(i):
        return np.float32(struct.unpack('<f', struct.pack('<I', i))[0])

    lo_i = f2i(np.float32(0.0))
    hi_i = f2i(np.float32(float(t) * float(t) * 4.0))
    while lo_i + 1 < hi_i:
        mid = (lo_i + hi_i) // 2
        if np.sqrt(i2f(mid), dtype=np.float32) <= t:
            lo_i = mid
        else:
            hi_i = mid
    return i2f(lo_i)


def _build_consts():
    v = _gauss_v()
    Bm = _band_matrix(H, {d - 2: v[d] for d in range(5)})
    BV2 = (Bm @ Bm).astype(np.float32)          # vertical gaussian applied twice
    blocks = []          # list of (t, s)
    mats = []
    for t in range(NB):
        for s in (t - 1, t, t + 1):
            if 0 <= s < NB:
                blk = BV2[128 * t:128 * (t + 1), 128 * s:128 * (s + 1)]
                blocks.append((t, s))
                mats.append(np.ascontiguousarray(blk.T))
    # sobel vertical operators as 128x128 blocks (same for every t)
    sv = np.zeros((128, 128), np.float32)        # s1[p] = g[p-1] + 2 g[p] + g[p+1]
    dv = np.zeros((128, 128), np.float32)        # d1[p] = g[p+1] - g[p-1]
    for p in range(128):
        sv[p, p] = 2.0
        if p > 0:
            sv[p, p - 1] = 1.0
            dv[p, p - 1] = -1.0
        if p < 127:
            sv[p, p + 1] = 1.0
            dv[p, p + 1] = 1.0
    svu = np.zeros((128, 128), np.float32); svu[0, 127] = 1.0     # from block t-1
    svd = np.zeros((128, 128), np.float32); svd[127, 0] = 1.0     # from block t+1
    dvu = np.zeros((128, 128), np.float32); dvu[0, 127] = -1.0
    dvd = np.zeros((128, 128), np.float32); dvd[127, 0] = 1.0
    shup = np.zeros((128, 128), np.float32)      # u[p] = x[p-1]
    shdn = np.zeros((128, 128), np.float32)      # d[p] = x[p+1]
    for p in range(128):
        if p > 0:
            shup[p, p - 1] = 1.0
        if p < 127:
            shdn[p, p + 1] = 1.0
    ident = np.eye(128, dtype=np.float32)
    extra_names = ['sv', 'svu', 'svd', 'dv', 'dvu', 'dvd', 'shup', 'shdn', 'ident']
    extra_idx = {}
    mats2 = []
    for nm, M in zip(extra_names, [sv, svu, svd, dv, dvu, dvd, shup, shdn, ident]):
        extra_idx[nm] = len(mats2)
        mats2.append(np.ascontiguousarray(M.T))
    bandT = np.ascontiguousarray(np.concatenate(mats, axis=1).astype(np.float32))
    band2T = np.ascontiguousarray(np.concatenate(mats2, axis=1).astype(np.float32))

    # horizontal gaussian^2 as matrix product C = Bw @ Bw (exact border rows/cols),
    # stored as a Toeplitz master strip + exact first/last block columns
    C64 = Bm @ Bm
    Cf = C64.astype(np.float32)
    w9 = np.array([C64[512, 512 + k - 4] for k in range(9)])
    masterS = np.zeros((128, 1152), np.float64)
    for p in range(128):
        lo = max(0, 512 + p - 4)
        for j in range(lo, min(1152, 512 + p + 5)):
            masterS[p, j] = w9[j - 512 - p + 4]
    masterS = masterS.astype(np.float32)
    for n in range(2):
        for vv in ([0, 1, 2, 3, 4] if n == 0 else [3, 4, 5, 6, 7]):
            if (vv, n) in [(0, 0), (7, 1)]:
                continue
            o = 512 + 512 * n - 128 * vv
            assert (Cf[128 * vv:128 * vv + 128, 512 * n:512 * n + 512]
                    == masterS[:, o:o + 512]).all()
    hcC = np.concatenate([masterS, Cf[0:128, 0:512], Cf[896:1024, 512:1024]],
                         axis=1)
    hcC = np.ascontiguousarray(hcC.astype(np.float32))

    # interior mask, packed: bit b of word (p, t*PW + j) is col 32j+b of row 128t+p
    interior = np.zeros((H, W), np.uint32)
    interior[1:-1, 1:-1] = 1
    ip = np.zeros((128, PF), np.uint32)
    for t in range(NB):
        rows = interior[128 * t:128 * (t + 1)]          # [128, W]
        bits = rows.reshape(128, PW, 32)
        words = (bits << np.arange(32, dtype=np.uint32)).sum(axis=2, dtype=np.uint32)
        ip[:, t * PW:(t + 1) * PW] = words
    ip = ip.view(np.int32)

    taps = [np.float32(x) for x in v]      # 5-tap horizontal gaussian
    consts = dict(
        bandT=bandT, band2T=band2T, hcC=hcC, blocks=blocks, extra_idx=extra_idx,
        interior_packed=np.ascontiguousarray(ip),
        taps=taps,
        KLOW=_threshold_sq(0.1), KHIGH=_threshold_sq(0.2),
        T1SQ=np.float32(np.tan(np.pi / 8) ** 2),
        T2SQ=np.float32(np.tan(3 * np.pi / 8) ** 2),
        zeros_f32=np.zeros((1, W), np.float32),
        zeros_i32=np.zeros((1, PW), np.int32),
    )
    return consts


# ---------------------------------------------------------------- helpers
def _stt(eng, out, in0, scalar, in1, op0, op1):
    from concourse import mybir as mb
    if isinstance(scalar, (int, np.integer)) and not isinstance(scalar, bool):
        imm = mb.ImmediateValue(dtype=mb.dt.int32, value=int(scalar))
    else:
        imm = mb.ImmediateValue(dtype=mb.dt.float32, value=float(scalar))
    return eng.add_instruction(
        mb.InstTensorScalarPtr(
            name=eng.bass.get_next_instruction_name(),
            is_scalar_tensor_tensor=True,
            op0=op0, op1=op1,
            ins=[eng.lower_ap(in0), imm, eng.lower_ap(in1)],
            outs=[eng.lower_ap(out)],
        ))


def _ts_int(eng, out, in0, s0, op0, s1=None, op1=None):
    from concourse import mybir as mb
    ins = [eng.lower_ap(in0), mb.ImmediateValue(dtype=mb.dt.int32, value=int(s0))]
    kw = dict(op0=op0)
    if s1 is not None:
        ins.append(mb.ImmediateValue(dtype=mb.dt.int32, value=int(s1)))
        kw['op1'] = op1
    return eng.add_instruction(
        mb.InstTensorScalarPtr(
            name=eng.bass.get_next_instruction_name(),
            ins=ins,
            outs=[eng.lower_ap(out)],
            **kw,
        ))


# ---------------------------------------------------------------- program
def build_program(debug=False):
    import concourse.tile as tile
    from concourse import bacc, mybir
    from contextlib import ExitStack
    dt = mybir.dt
    op = mybir.AluOpType
    AF = mybir.ActivationFunctionType
    C = _build_consts()
    EI = C['extra_idx']

    nc = bacc.Bacc("TRN2", target_bir_lowering=False, debug=False)
    nblk = C['bandT'].shape[1] // 128
    nblk2 = C['band2T'].shape[1] // 128
    x_d = nc.dram_tensor("x", [3, H, W], dt.float32, kind="ExternalInput").ap()
    band_d = nc.dram_tensor("bandT", [128, nblk * 128], dt.float32, kind="ExternalInput").ap()
    band2_d = nc.dram_tensor("band2T", [128, nblk2 * 128], dt.float32, kind="ExternalInput").ap()
    hc_d = nc.dram_tensor("hcC", [128, 2176], dt.float32, kind="ExternalInput").ap()
    ip_d = nc.dram_tensor("interior", [128, PF], dt.int32, kind="ExternalInput").ap()
    zf_d = nc.dram_tensor("zeros_f32", [1, W], dt.float32, kind="ExternalInput").ap()
    zi_d = nc.dram_tensor("zeros_i32", [1, PW], dt.int32, kind="ExternalInput").ap()
    out_d = nc.dram_tensor("out", [H, W], dt.float32, kind="ExternalOutput").ap()
    dbg = {}
    if debug:
        for name in ("gray", "g1", "gfull", "gx", "gy", "m2", "kmask", "low0", "e0"):
            dbg[name] = nc.dram_tensor("dbg_" + name, [128, FF], dt.float32,
                                       kind="ExternalOutput").ap()
        for name in ("lowp", "e0p", "ep"):
            dbg[name] = nc.dram_tensor("dbg_" + name, [128, PF], dt.int32,
                                       kind="ExternalOutput").ap()

    with tile.TileContext(nc) as tc, ExitStack() as ctx:
        pool = ctx.enter_context(tc.tile_pool(name="main", bufs=1))
        psA = ctx.enter_context(tc.tile_pool(name="psA", bufs=1, space="PSUM"))
        psB = ctx.enter_context(tc.tile_pool(name="psB", bufs=1, space="PSUM"))

        def f32buf(tag, name):
            return pool.tile([128, FF], dt.float32, tag=tag, name=name)

        def blk(buf, t, n=None):
            if n is None:
                return buf[:, W * t:W * (t + 1)]
            return buf[:, W * t + 512 * n: W * t + 512 * (n + 1)]

        band = pool.tile([128, nblk * 128], dt.float32, tag="Ct", name="band")
        nc.sync.dma_start(band[:], band_d[:])
        band2 = pool.tile([128, nblk2 * 128], dt.float32, tag="band2", name="band2")
        nc.scalar.dma_start(band2[:], band2_d[:])
        ipm = pool.tile([128, PF], dt.int32, tag="ipm", name="ipm")
        nc.scalar.dma_start(ipm[:], ip_d[:])

        def bmat(i):
            return band[:, 128 * i:128 * (i + 1)]

        def bmat2(i):
            return band2[:, 128 * i:128 * (i + 1)]

        # dummy matmul to absorb the const-DMA semaphore on PE early
        dps = psB.tile([128, 128], dt.float32, tag="mmD1", name="dummy", bufs=2)
        nc.tensor.matmul(dps[:], bmat(0), bmat(0), start=True, stop=True)

        # ---------------- gray ---------------------------------------------
        A = f32buf("A", "gray")
        for t in range(NB):
            r = pool.tile([128, W], dt.float32, tag="chR", name="chR")
            g = pool.tile([128, W], dt.float32, tag="chG", name="chG")
            b = pool.tile([128, W], dt.float32, tag="chB", name="chB")
            nc.sync.dma_start(r[:], x_d[0, 128 * t:128 * (t + 1), :])
            nc.scalar.dma_start(g[:], x_d[1, 128 * t:128 * (t + 1), :])
            nc.sync.dma_start(b[:], x_d[2, 128 * t:128 * (t + 1), :])
            sl = blk(A, t)
            nc.scalar.activation(sl, r[:], AF.Copy, scale=0.299)
            _stt(nc.vector, sl, g[:], 0.587, sl, op.mult, op.add)
            _stt(nc.vector, sl, b[:], 0.114, sl, op.mult, op.add)
        if debug:
            nc.sync.dma_start(dbg["gray"][:], A[:])

        # ---------------- vertical gaussian^2 on PE -> Bt -------------------
        Bb = f32buf("Bt", "g1")
        bmap = {}
        for i, (t, s) in enumerate(C['blocks']):
            bmap.setdefault(t, []).append((s, i))
        for t in range(NB):
            for n in range(2):
                ps = psA.tile([128, 512], dt.float32, tag="mmB", name="mmB", bufs=2)
                lst = bmap[t]
                for j, (s, i) in enumerate(lst):
                    nc.tensor.matmul(ps[:], bmat(i), blk(A, s, n),
                                     start=(j == 0), stop=(j == len(lst) - 1))
                nc.scalar.activation(blk(Bb, t, n), ps[:], AF.Copy)
        if debug:
            nc.sync.dma_start(dbg["g1"][:], Bb[:])

        # ---------------- horizontal gaussian (5-tap, twice) ----------------
        taps = C['taps']

        def hconv5(dst, src):
            # out[j] = sum_d v[d] src[j+d-2], zero padded, per 1024-block
            for t in range(NB):
                a = W * t
                ve = nc.vector
                nc.scalar.activation(dst[:, a:a + W], src[:, a:a + W], AF.Copy,
                                     scale=float(taps[2]))
                _stt(ve, dst[:, a + 2:a + W], src[:, a:a + W - 2], float(taps[0]),
                     dst[:, a + 2:a + W], op.mult, op.add)
                _stt(ve, dst[:, a + 1:a + W], src[:, a:a + W - 1], float(taps[1]),
                     dst[:, a + 1:a + W], op.mult, op.add)
                _stt(ve, dst[:, a:a + W - 1], src[:, a + 1:a + W], float(taps[3]),
                     dst[:, a:a + W - 1], op.mult, op.add)
                _stt(ve, dst[:, a:a + W - 2], src[:, a + 2:a + W], float(taps[4]),
                     dst[:, a:a + W - 2], op.mult, op.add)

        # g_full = g1 @ C on PE: per row-tile, transpose 128x128 subtiles of g1
        # (PE transpose mode), then banded matmuls against the C master strip.
        hcC = pool.tile([128, 2176], dt.float32, tag="Dt", name="hcC")
        nc.sync.dma_start(hcC[:], hc_d[:])
        GF = A              # gray dead; g_full lands in A
        def _hp(db, doff, sb, soff):
            # one 5-tap pass on a 1024 block: db[doff:] = BW(sb[soff:])
            nc.scalar.activation(db[:, doff:doff + W], sb[:, soff:soff + W],
                                 AF.Copy, scale=float(taps[2]))
            _stt(nc.vector, db[:, doff + 2:doff + W], sb[:, soff:soff + W - 2],
                 float(taps[0]), db[:, doff + 2:doff + W], op.mult, op.add)
            _stt(nc.vector, db[:, doff + 1:doff + W], sb[:, soff:soff + W - 1],
                 float(taps[1]), db[:, doff + 1:doff + W], op.mult, op.add)
            _stt(nc.vector, db[:, doff:doff + W - 1], sb[:, soff + 1:soff + W],
                 float(taps[3]), db[:, doff:doff + W - 1], op.mult, op.add)
            _stt(nc.vector, db[:, doff:doff + W - 2], sb[:, soff + 2:soff + W],
                 float(taps[4]), db[:, doff:doff + W - 2], op.mult, op.add)

        for t in range(4, NB):     # DVE path for blocks 4..7 (overlaps PE blocks)
            a = W * t
            tmp = pool.tile([128, W], dt.float32, tag="chG", name="htmp")
            _hp(tmp, 0, Bb, a)
            _hp(GF, a, tmp, 0)
        for t in range(4):
            g1T = pool.tile([128, 1024], dt.float32, tag="chR", name="g1T")
            for k in range(8):
                pst = psB.tile([128, 128], dt.float32, tag="mmD1", name="tp", bufs=2)
                nc.tensor.transpose(pst[:], Bb[:, W * t + 128 * k: W * t + 128 * (k + 1)],
                                    bmat2(EI['ident']))
                nc.scalar.activation(g1T[:, 128 * k:128 * (k + 1)], pst[:], AF.Copy)
            for n in range(2):
                pso = psA.tile([128, 512], dt.float32, tag="mmB", name="hco", bufs=2)
                vs = [0, 1, 2, 3, 4] if n == 0 else [3, 4, 5, 6, 7]
                for j, vv in enumerate(vs):
                    if (vv, n) == (0, 0):
                        rhs = hcC[:, 1152:1664]
                    elif (vv, n) == (7, 1):
                        rhs = hcC[:, 1664:2176]
                    else:
                        o = 512 + 512 * n - 128 * vv
                        rhs = hcC[:, o:o + 512]
                    nc.tensor.matmul(pso[:], g1T[:, 128 * vv:128 * (vv + 1)], rhs,
                                     start=(j == 0), stop=(j == len(vs) - 1))
                nc.scalar.activation(blk(GF, t, n), pso[:], AF.Copy)
        if debug:
            nc.sync.dma_start(dbg["gfull"][:], GF[:])

        # ---------------- sobel vertical parts: u/d row shifts on PE ---------
        # u[p] = g[p-1], d[p] = g[p+1]; s1 = u + 2g + d ; d1 = d - u (DVE)
        U = f32buf("Bt", "ush")      # g1 dead after hconv
        Dd = f32buf("Ct", "dsh")     # band1 dead
        for t in range(NB):
            for n in range(2):
                ps = psA.tile([128, 512], dt.float32, tag="mmS", name="mmS", bufs=2)
                nc.tensor.matmul(ps[:], bmat2(EI['shup']), blk(GF, t, n),
                                 start=True, stop=True)
                nc.scalar.activation(blk(U, t, n), ps[:], AF.Copy)
                ps2 = psB.tile([128, 512], dt.float32, tag="mmD1", name="mmD1", bufs=2)
                nc.tensor.matmul(ps2[:], bmat2(EI['shdn']), blk(GF, t, n),
                                 start=True, stop=True)
                nc.scalar.activation(blk(Dd, t, n), ps2[:], AF.Copy)
        # block-crossing boundary rows
        nc.sync.dma_start(U[0:1, W:FF], GF[127:128, 0:FF - W])
        nc.vector.memset(U[0:1, 0:W], 0.0)
        nc.scalar.dma_start(Dd[127:128, 0:FF - W], GF[0:1, W:FF])
        nc.scalar.dma_start(Dd[127:128, FF - W:FF], zf_d[:])
        D1 = f32buf("Dt", "d1")      # hcC dead
        nc.vector.tensor_tensor(D1[:], Dd[:], U[:], op.subtract)
        _stt(nc.vector, U[:], GF[:], 2.0, U[:], op.mult, op.add)
        nc.vector.tensor_tensor(U[:], U[:], Dd[:], op.add)
        S1 = U      # s1 now lives in Bt

        # gx = D_W(s1) -> Ct ; gy = S_W(d1) -> Et
        GX = Dd
        GY = f32buf("Et", "gy")
        for t in range(NB):
            a = W * t
            ve = nc.vector
            # gx[j] = s1[j+1] - s1[j-1]
            nc.scalar.activation(GX[:, a:a + W - 1], S1[:, a + 1:a + W], AF.Copy)
            nc.vector.memset(GX[:, a + W - 1:a + W], 0.0)
            _stt(ve, GX[:, a + 1:a + W], S1[:, a:a + W - 1], -1.0,
                 GX[:, a + 1:a + W], op.mult, op.add)
            # gy[j] = d1[j-1] + 2 d1[j] + d1[j+1]
            nc.scalar.activation(GY[:, a:a + W], D1[:, a:a + W], AF.Copy, scale=2.0)
            _stt(ve, GY[:, a + 1:a + W], D1[:, a:a + W - 1], 1.0,
                 GY[:, a + 1:a + W], op.mult, op.add)
            _stt(ve, GY[:, a:a + W - 1], D1[:, a + 1:a + W], 1.0,
                 GY[:, a:a + W - 1], op.mult, op.add)
        if debug:
            nc.sync.dma_start(dbg["gx"][:], GX[:])
            nc.sync.dma_start(dbg["gy"][:], GY[:])

        # ---------------- m2 / direction masks ------------------------------
        M2X = A             # g_full dead after sobel PE
        M2Y = D1            # d1 dead after gy
        nc.scalar.activation(M2X[:], GX[:], AF.Square)
        nc.scalar.activation(M2Y[:], GY[:], AF.Square)
        c0 = pool.tile([128, FF], dt.uint8, tag="haa", name="c0")
        c2 = pool.tile([128, FF], dt.uint8, tag="hcc", name="c2")
        c1 = pool.tile([128, FF], dt.uint8, tag="hbb", name="c1")
        _stt(nc.vector, c0[:], M2X[:], float(C['T1SQ']), M2Y[:], op.mult, op.is_ge)
        _stt(nc.vector, c2[:], M2X[:], float(C['T2SQ']), M2Y[:], op.mult, op.is_le)
        M2 = S1             # s1 dead after gx (Bt slot)
        nc.gpsimd.tensor_tensor(M2[:], M2X[:], M2Y[:], op.add)
        PROD = GX           # overwrite gx in place
        nc.vector.tensor_tensor(PROD[:], GX[:], GY[:], op.mult)
        # c1 = (prod >= 0) & !(c0 | c2)
        nc.vector.tensor_tensor(c1[:], c0[:], c2[:], op.logical_or)
        nc.vector.tensor_scalar(c1[:], c1[:], 0.0, None, op.is_equal)
        _stt(nc.vector, c1[:], PROD[:], 0.0, c1[:], op.is_ge, op.logical_and)
        if debug:
            nc.sync.dma_start(dbg["m2"][:], M2[:])

        # ---------------- NMS ------------------------------------------------
        # m2u = m2[row-1] -> M2X slot ; m2d = m2[row+1] -> M2Y slot (PE shifts)
        M2U = A
        M2D = D1
        for t in range(NB):
            for n in range(2):
                ps = psA.tile([128, 512], dt.float32, tag="mmU", name="mmU", bufs=1)
                nc.tensor.matmul(ps[:], bmat2(EI['shup']), blk(M2, t, n),
                                 start=True, stop=True)
                nc.scalar.activation(blk(M2U, t, n), ps[:], AF.Copy)
                ps2 = psB.tile([128, 512], dt.float32, tag="mmV", name="mmV", bufs=1)
                nc.tensor.matmul(ps2[:], bmat2(EI['shdn']), blk(M2, t, n),
                                 start=True, stop=True)
                nc.scalar.activation(blk(M2D, t, n), ps2[:], AF.Copy)
        # boundary rows across blocks
        nc.sync.dma_start(M2U[0:1, W:FF], M2[127:128, 0:FF - W])
        nc.vector.memset(M2U[0:1, 0:W], 0.0)
        nc.scalar.dma_start(M2D[127:128, 0:FF - W], M2[0:1, W:FF])
        nc.scalar.dma_start(M2D[127:128, FF - W:FF], zf_d[:])

        km = GY             # gy dead after prod
        scrtags = ["chR", "chG", "chB"]

        def nms_cat(catmask, a1, sh1, a2, sh2, first, ci):
            for t in range(NB):
                a = W * t
                lo = max(0, -sh1, -sh2)
                hi = min(W, W - sh1, W - sh2)
                tg = scrtags[(ci * NB + t) % 3]
                eng = nc.vector
                scr = pool.tile([128, W], dt.float32, tag=tg, name="scr")
                eng.tensor_tensor(scr[:, lo:hi],
                                  a1[:, a + lo + sh1:a + hi + sh1],
                                  a2[:, a + lo + sh2:a + hi + sh2], op.max)
                eng.tensor_tensor(scr[:, lo:hi], M2[:, a + lo:a + hi],
                                  scr[:, lo:hi], op.is_ge)
                if first:
                    nc.vector.tensor_copy(km[:, a + lo:a + hi], scr[:, lo:hi])
                    if lo > 0:
                        nc.vector.memset(km[:, a:a + lo], 0.0)
                        nc.vector.memset(km[:, a + hi:a + W], 0.0)
                else:
                    nc.vector.copy_predicated(km[:, a + lo:a + hi],
                                              catmask[:, a + lo:a + hi],
                                              scr[:, lo:hi])

        nms_cat(None, M2U, -1, M2D, 1, True, 0)     # cat3
        nms_cat(c2, M2U, 0, M2D, 0, False, 1)       # cat2
        nms_cat(c1, M2U, 1, M2D, -1, False, 2)      # cat1
        nms_cat(c0, M2, -1, M2, 1, False, 3)        # cat0
        if debug:
            nc.sync.dma_start(dbg["kmask"][:], km[:])

        # ---------------- thresholds -> f32 0/1 ------------------------------
        lowf = GX            # prod dead
        e0f = M2D            # m2d dead after cat1
        _stt(nc.vector, lowf[:], M2[:], float(C['KLOW']), km[:], op.is_gt, op.mult)
        _stt(nc.vector, e0f[:], M2[:], float(C['KHIGH']), km[:], op.is_gt, op.mult)
        if debug:
            nc.sync.dma_start(dbg["low0"][:], lowf[:])
            nc.sync.dma_start(dbg["e0"][:], e0f[:])

        # ---------------- pack to bits ---------------------------------------
        pia = pool.tile([128, FF], dt.int32, tag="A", name="pia")
        pib = pool.tile([128, FF], dt.int32, tag="Et", name="pib")

        def pack(dstp, srcf):
            nc.vector.tensor_copy(pia[:], srcf[:])          # f32 0/1 -> int32
            cur = pia[:]
            n = FF
            for lvl in range(5):
                n //= 2
                sh = 1 << lvl
                vw = cur.rearrange("p (n two) -> p n two", two=2)
                if lvl == 4:
                    dst = dstp[:]
                elif lvl % 2 == 0:
                    dst = pib[:, 0:n]
                else:
                    dst = pia[:, 0:n]
                _stt(nc.vector, dst, vw[:, :, 1], sh, vw[:, :, 0],
                     op.logical_shift_left, op.bitwise_or)
                cur = dst

        lowp = pool.tile([128, PF], dt.int32, tag="lp2", name="lowp")
        e0p = pool.tile([128, PF], dt.int32, tag="ep2", name="e0p")
        pack(lowp, lowf)
        pack(e0p, e0f)
        nc.vector.tensor_tensor(lowp[:], lowp[:], ipm[:], op.bitwise_and)
        nc.vector.tensor_tensor(e0p[:], e0p[:], ipm[:], op.bitwise_and)
        if debug:
            nc.sync.dma_start(dbg["lowp"][:], lowp[:])
            nc.sync.dma_start(dbg["e0p"][:], e0p[:])

        # ---------------- hysteresis flood fill ------------------------------
        e = e0p
        aa = pool.tile([128, PF], dt.int32, tag="haa", name="haa")
        bb2 = pool.tile([128, PF], dt.int32, tag="hbb", name="hbb")
        cc = pool.tile([128, PF], dt.int32, tag="hcc", name="hcc")
        dup = pool.tile([128, PF], dt.int32, tag="hdup", name="hdup")
        ddn = pool.tile([128, PF], dt.int32, tag="hddn", name="hddn")
        av = aa.rearrange("p (n w) -> p n w", w=PW)
        bv = bb2.rearrange("p (n w) -> p n w", w=PW)
        cv = cc.rearrange("p (n w) -> p n w", w=PW)
        mask_up = [min(i + 1, 31) for i in range(32)]   # dup[p] = c[p+1]
        mask_dn = [max(i - 1, 0) for i in range(32)]    # ddn[p] = c[p-1]
        for it in range(N_ITER):
            # horizontal dilate with cross-word carries
            _stt(nc.vector, aa[:], e[:], 1, e[:], op.logical_shift_left, op.bitwise_or)
            _stt(nc.vector, aa[:], e[:], 1, aa[:], op.logical_shift_right, op.bitwise_or)
            _stt(nc.vector, bb2[:, 1:PF], e[:, 0:PF - 1], 31, aa[:, 1:PF],
                 op.logical_shift_right, op.bitwise_or)
            nc.vector.tensor_copy(bv[:, :, 0], av[:, :, 0])
            _stt(nc.vector, cc[:, 0:PF - 1], e[:, 1:PF], 31, bb2[:, 0:PF - 1],
                 op.logical_shift_left, op.bitwise_or)
            nc.vector.tensor_copy(cv[:, :, PW - 1], bv[:, :, PW - 1])
            # vertical neighbors via stream_shuffle + boundary DMAs
            nc.vector.stream_shuffle(dup[:], cc[:], mask_up)
            nc.vector.stream_shuffle(ddn[:], cc[:], mask_dn)
            q1 = nc.sync if it % 2 == 0 else nc.scalar
            q2 = nc.scalar if it % 2 == 0 else nc.sync
            # rows masked to zero by lowp's interior mask need no zero fill
            q1.dma_start(dup[31:127:32, :], cc[32:128:32, :])
            q1.dma_start(dup[127:128, 0:PF - PW], cc[0:1, PW:PF])
            q2.dma_start(ddn[32:128:32, :], cc[31:127:32, :])
            q2.dma_start(ddn[0:1, PW:PF], cc[127:128, 0:PF - PW])
            # e' = (dup | ddn | c) & low
            nc.vector.tensor_tensor(dup[:], dup[:], ddn[:], op.bitwise_or)
            nc.vector.tensor_tensor(dup[:], dup[:], cc[:], op.bitwise_or)
            nc.vector.tensor_tensor(e[:], dup[:], lowp[:], op.bitwise_and)
        if debug:
            nc.sync.dma_start(dbg["ep"][:], e[:])

        # ---------------- unpack complement -> output ------------------------
        ne = aa
        _stt(nc.vector, ne[:], e[:], -1, e[:], op.bitwise_xor, op.bypass)
        outf = f32buf("A", "outf")
        cur = ne[:]
        n = PF
        masks = [0xFFFF, 0xFF, 0xF, 0x3, 0x1]
        for lvl in range(5):
            half = 32 >> (lvl + 1)       # bits per half-word after split
            m = masks[lvl]
            dst = (pib[:, 0:2 * n] if lvl % 2 == 0 else pia[:, 0:2 * n])
            dv2 = dst.rearrange("p (n two) -> p n two", two=2)
            _ts_int(nc.vector, dv2[:, :, 0], cur, m, op.bitwise_and)
            _ts_int(nc.vector, dv2[:, :, 1], cur, half, op.logical_shift_right,
                    m, op.bitwise_and)
            cur = dst
            n *= 2
        nc.vector.tensor_copy(outf[:], pib[:])    # int 0/1 -> f32
        for t in range(NB):
            q = nc.sync if t % 2 == 0 else nc.scalar
            q.dma_start(out_d[128 * t:128 * (t + 1), :], outf[:, W * t:W * (t + 1)])

    nc.compile()
    return nc, C, dbg


def _run(inputs, debug=False, trace=False):
    from concourse.bass_utils import run_bass_kernel_spmd
    key = ("dbg" if debug else "plain")
    if key not in _cache:
        _cache[key] = build_program(debug=debug)
    nc, C, dbg = _cache[key]
    x = np.asarray(inputs["x"], dtype=np.float32)
    in_maps = []
    for c in range(B):
        in_maps.append({
            "x": np.ascontiguousarray(x[c]),
            "bandT": C['bandT'],
            "band2T": C['band2T'],
            "hcC": C['hcC'],
            "interior": C['interior_packed'],
            "zeros_f32": C['zeros_f32'],
            "zeros_i32": C['zeros_i32'],
        })
    res = run_bass_kernel_spmd(nc, in_maps, core_ids=list(range(B)), trace=trace)
    return res


def kernel(x, gaussian_kernel=None, sobel_x=None, sobel_y=None):
    res = _run({"x": x})
    out = np.stack([res.results[c]["out"] for c in range(B)], axis=0)
    return out.reshape(B, 1, H, W).astype(np.float32)


# revision 23
# speedup vs baseline: 1.1373x; 1.0107x over previous
"""Canny edge detector on 8 Trainium2 NeuronCores — pure data parallel,
one 1024x1024 image per core.

Per-core pipeline (all decisions in f32, no sqrt/atan2 anywhere):
  gray -> vertical gaussian^2 (PE banded f32 matmuls) -> horizontal
  gaussian (2x 5-tap fused STT passes) -> sobel vertical parts as PE banded
  matmuls, horizontal parts as free-dim taps -> squared-magnitude NMS with
  copy_predicated direction select (row-shifted squared magnitudes via PE
  shift matmuls) -> thresholds on squared magnitude -> 32x bit-packed
  hysteresis flood fill (fused shift/or int passes + stream_shuffle row
  shifts with small DMA boundary fixes).
"""
import math
import numpy as np

B, H, W = 8, 1024, 1024
NB = H // 128          # 8 row blocks
PW = W // 32           # 32 packed words per row per block
PF = NB * PW           # 256 packed words per partition
FF = NB * W            # 8192 f32 elems per partition
N_ITER = 17

_cache = {}


# ---------------------------------------------------------------- constants
def _gauss_v():
    x = np.linspace(-2, 2, 5).astype(np.float64)
    g2 = np.exp(-(x.reshape(5, 1) ** 2 + x.reshape(1, 5) ** 2) / 2.0)
    K = g2 / g2.sum()
    v = K[:, 2] / math.sqrt(K[2, 2])
    return v  # 5-tap 1D gaussian, outer(v,v) = 2D kernel


def _band_matrix(n, taps):
    M = np.zeros((n, n), dtype=np.float64)
    for d, w in taps.items():
        i = np.arange(n)
        j = i + d
        m = (j >= 0) & (j < n)
        M[i[m], j[m]] = w
    return M


def _threshold_sq(t):
    import struct
    t = np.float32(t)

    def f2i(f):
        return struct.unpack('<I', struct.pack('<f', np.float32(f)))[0]

    def i2f(i):
        return np.float32(struct.unpack('<f', struct.pack('<I', i))[0])

    lo_i = f2i(np.float32(0.0))
    hi_i = f2i(np.float32(float(t) * float(t) * 4.0))
    while lo_i + 1 < hi_i:
        mid = (lo_i + hi_i) // 2
        if np.sqrt(i2f(mid), dtype=np.float32) <= t:
            lo_i = mid
        else:
            hi_i = mid
    return i2f(lo_i)


def _build_consts():
    v = _gauss_v()
    Bm = _band_matrix(H, {d - 2: v[d] for d in range(5)})
    BV2 = (Bm @ Bm).astype(np.float32)          # vertical gaussian applied twice
    blocks = []          # list of (t, s)
    mats = []
    for t in range(NB):
        for s in (t - 1, t, t + 1):
            if 0 <= s < NB:
                blk = BV2[128 * t:128 * (t + 1), 128 * s:128 * (s + 1)]
                blocks.append((t, s))
                mats.append(np.ascontiguousarray(blk.T))
    # sobel vertical operators as 128x128 blocks (same for every t)
    sv = np.zeros((128, 128), np.float32)        # s1[p] = g[p-1] + 2 g[p] + g[p+1]
    dv = np.zeros((128, 128), np.float32)        # d1[p] = g[p+1] - g[p-1]
    for p in range(128):
        sv[p, p] = 2.0
        if p > 0:
            sv[p, p - 1] = 1.0
            dv[p, p - 1] = -1.0
        if p < 127:
            sv[p, p + 1] = 1.0
            dv[p, p + 1] = 1.0
    svu = np.zeros((128, 128), np.float32); svu[0, 127] = 1.0     # from block t-1
    svd = np.zeros((128, 128), np.float32); svd[127, 0] = 1.0     # from block t+1
    dvu = np.zeros((128, 128), np.float32); dvu[0, 127] = -1.0
    dvd = np.zeros((128, 128), np.float32); dvd[127, 0] = 1.0
    shup = np.zeros((128, 128), np.float32)      # u[p] = x[p-1]
    shdn = np.zeros((128, 128), np.float32)      # d[p] = x[p+1]
    for p in range(128):
        if p > 0:
            shup[p, p - 1] = 1.0
        if p < 127:
            shdn[p, p + 1] = 1.0
    ident = np.eye(128, dtype=np.float32)
    extra_names = ['sv', 'svu', 'svd', 'dv', 'dvu', 'dvd', 'shup', 'shdn', 'ident']
    extra_idx = {}
    mats2 = []
    for nm, M in zip(extra_names, [sv, svu, svd, dv, dvu, dvd, shup, shdn, ident]):
        extra_idx[nm] = len(mats2)
        mats2.append(np.ascontiguousarray(M.T))
    bandT = np.ascontiguousarray(np.concatenate(mats, axis=1).astype(np.float32))
    band2T = np.ascontiguousarray(np.concatenate(mats2, axis=1).astype(np.float32))

    # horizontal gaussian^2 as matrix product C = Bw @ Bw (exact border rows/cols),
    # stored as a Toeplitz master strip + exact first/last block columns
    C64 = Bm @ Bm
    Cf = C64.astype(np.float32)
    w9 = np.array([C64[512, 512 + k - 4] for k in range(9)])
    masterS = np.zeros((128, 1152), np.float64)
    for p in range(128):
        lo = max(0, 512 + p - 4)
        for j in range(lo, min(1152, 512 + p + 5)):
            masterS[p, j] = w9[j - 512 - p + 4]
    masterS = masterS.astype(np.float32)
    for n in range(2):
        for vv in ([0, 1, 2, 3, 4] if n == 0 else [3, 4, 5, 6, 7]):
            if (vv, n) in [(0, 0), (7, 1)]:
                continue
            o = 512 + 512 * n - 128 * vv
            assert (Cf[128 * vv:128 * vv + 128, 512 * n:512 * n + 512]
                    == masterS[:, o:o + 512]).all()
    hcC = np.concatenate([masterS, Cf[0:128, 0:512], Cf[896:1024, 512:1024]],
                         axis=1)
    hcC = np.ascontiguousarray(hcC.astype(np.float32))

    # interior mask, packed: bit b of word (p, t*PW + j) is col 32j+b of row 128t+p
    interior = np.zeros((H, W), np.uint32)
    interior[1:-1, 1:-1] = 1
    ip = np.zeros((128, PF), np.uint32)
    for t in range(NB):
        rows = interior[128 * t:128 * (t + 1)]          # [128, W]
        bits = rows.reshape(128, PW, 32)
        words = (bits << np.arange(32, dtype=np.uint32)).sum(axis=2, dtype=np.uint32)
        ip[:, t * PW:(t + 1) * PW] = words
    ip = ip.view(np.int32)

    taps = [np.float32(x) for x in v]      # 5-tap horizontal gaussian
    consts = dict(
        bandT=bandT, band2T=band2T, hcC=hcC, blocks=blocks, extra_idx=extra_idx,
        interior_packed=np.ascontiguousarray(ip),
        taps=taps,
        KLOW=_threshold_sq(0.1), KHIGH=_threshold_sq(0.2),
        T1SQ=np.float32(np.tan(np.pi / 8) ** 2),
        T2SQ=np.float32(np.tan(3 * np.pi / 8) ** 2),
        zeros_f32=np.zeros((1, W), np.float32),
        zeros_i32=np.zeros((1, PW), np.int32),
    )
    return consts


# ---------------------------------------------------------------- helpers
def _stt(eng, out, in0, scalar, in1, op0, op1):
    from concourse import mybir as mb
    if isinstance(scalar, (int, np.integer)) and not isinstance(scalar, bool):
        imm = mb.ImmediateValue(dtype=mb.dt.int32, value=int(scalar))
    else:
        imm = mb.ImmediateValue(dtype=mb.dt.float32, value=float(scalar))
    return eng.add_instruction(
        mb.InstTensorScalarPtr(
            name=eng.bass.get_next_instruction_name(),
            is_scalar_tensor_tensor=True,
            op0=op0, op1=op1,
            ins=[eng.lower_ap(in0), imm, eng.lower_ap(in1)],
            outs=[eng.lower_ap(out)],
        ))


def _ts_int(eng, out, in0, s0, op0, s1=None, op1=None):
    from concourse import mybir as mb
    ins = [eng.lower_ap(in0), mb.ImmediateValue(dtype=mb.dt.int32, value=int(s0))]
    kw = dict(op0=op0)
    if s1 is not None:
        ins.append(mb.ImmediateValue(dtype=mb.dt.int32, value=int(s1)))
        kw['op1'] = op1
    return eng.add_instruction(
        mb.InstTensorScalarPtr(
            name=eng.bass.get_next_instruction_name(),
            ins=ins,
            outs=[eng.lower_ap(out)],
            **kw,
        ))


# ---------------------------------------------------------------- program
def build_program(debug=False):
    import concourse.tile as tile
    from concourse import bacc, mybir
    from contextlib import ExitStack
    dt = mybir.dt
    op = mybir.AluOpType
    AF = mybir.ActivationFunctionType
    C = _build_consts()
    EI = C['extra_idx']

    nc = bacc.Bacc("TRN2", target_bir_lowering=False, debug=False)
    nblk = C['bandT'].shape[1] // 128
    nblk2 = C['band2T'].shape[1] // 128
    x_d = nc.dram_tensor("x", [3, H, W], dt.float32, kind="ExternalInput").ap()
    band_d = nc.dram_tensor("bandT", [128, nblk * 128], dt.float32, kind="ExternalInput").ap()
    band2_d = nc.dram_tensor("band2T", [128, nblk2 * 128], dt.float32, kind="ExternalInput").ap()
    hc_d = nc.dram_tensor("hcC", [128, 2176], dt.float32, kind="ExternalInput").ap()
    ip_d = nc.dram_tensor("interior", [128, PF], dt.int32, kind="ExternalInput").ap()
    zf_d = nc.dram_tensor("zeros_f32", [1, W], dt.float32, kind="ExternalInput").ap()
    zi_d = nc.dram_tensor("zeros_i32", [1, PW], dt.int32, kind="ExternalInput").ap()
    out_d = nc.dram_tensor("out", [H, W], dt.float32, kind="ExternalOutput").ap()
    dbg = {}
    if debug:
        for name in ("gray", "g1", "gfull", "gx", "gy", "m2", "kmask", "low0", "e0"):
            dbg[name] = nc.dram_tensor("dbg_" + name, [128, FF], dt.float32,
                                       kind="ExternalOutput").ap()
        for name in ("lowp", "e0p", "ep"):
            dbg[name] = nc.dram_tensor("dbg_" + name, [128, PF], dt.int32,
                                       kind="ExternalOutput").ap()

    with tile.TileContext(nc) as tc, ExitStack() as ctx:
        pool = ctx.enter_context(tc.tile_pool(name="main", bufs=1))
        psA = ctx.enter_context(tc.tile_pool(name="psA", bufs=1, space="PSUM"))
        psB = ctx.enter_context(tc.tile_pool(name="psB", bufs=1, space="PSUM"))

        def f32buf(tag, name):
            return pool.tile([128, FF], dt.float32, tag=tag, name=name)

        def blk(buf, t, n=None):
            if n is None:
                return buf[:, W * t:W * (t + 1)]
            return buf[:, W * t + 512 * n: W * t + 512 * (n + 1)]

        band = pool.tile([128, nblk * 128], dt.float32, tag="Ct", name="band")
        nc.sync.dma_start(band[:], band_d[:])
        band2 = pool.tile([128, nblk2 * 128], dt.float32, tag="band2", name="band2")
        nc.scalar.dma_start(band2[:], band2_d[:])
        ipm = pool.tile([128, PF], dt.int32, tag="ipm", name="ipm")
        nc.scalar.dma_start(ipm[:], ip_d[:])

        def bmat(i):
            return band[:, 128 * i:128 * (i + 1)]

        def bmat2(i):
            return band2[:, 128 * i:128 * (i + 1)]

        # dummy matmul to absorb the const-DMA semaphore on PE early
        dps = psB.tile([128, 128], dt.float32, tag="mmD1", name="dummy", bufs=2)
        nc.tensor.matmul(dps[:], bmat(0), bmat(0), start=True, stop=True)

        # ---------------- gray ---------------------------------------------
        A = f32buf("A", "gray")
        for t in range(NB):
            r = pool.tile([128, W], dt.float32, tag="chR", name="chR")
            g = pool.tile([128, W], dt.float32, tag="chG", name="chG")
            b = pool.tile([128, W], dt.float32, tag="chB", name="chB")
            nc.sync.dma_start(r[:], x_d[0, 128 * t:128 * (t + 1), :])
            nc.scalar.dma_start(g[:], x_d[1, 128 * t:128 * (t + 1), :])
            nc.sync.dma_start(b[:], x_d[2, 128 * t:128 * (t + 1), :])
            sl = blk(A, t)
            nc.scalar.activation(sl, r[:], AF.Copy, scale=0.299)
            _stt(nc.vector, sl, g[:], 0.587, sl, op.mult, op.add)
            _stt(nc.vector, sl, b[:], 0.114, sl, op.mult, op.add)
        if debug:
            nc.sync.dma_start(dbg["gray"][:], A[:])

        # ---------------- vertical gaussian^2 on PE -> Bt -------------------
        Bb = f32buf("Bt", "g1")
        bmap = {}
        for i, (t, s) in enumerate(C['blocks']):
            bmap.setdefault(t, []).append((s, i))
        for t in range(NB):
            for n in range(2):
                ps = psA.tile([128, 512], dt.float32, tag="mmB", name="mmB", bufs=2)
                lst = bmap[t]
                for j, (s, i) in enumerate(lst):
                    nc.tensor.matmul(ps[:], bmat(i), blk(A, s, n),
                                     start=(j == 0), stop=(j == len(lst) - 1))
                nc.scalar.activation(blk(Bb, t, n), ps[:], AF.Copy)
        if debug:
            nc.sync.dma_start(dbg["g1"][:], Bb[:])

        # ---------------- horizontal gaussian (5-tap, twice) ----------------
        taps = C['taps']

        def hconv5(dst, src):
            # out[j] = sum_d v[d] src[j+d-2], zero padded, per 1024-block
            for t in range(NB):
                a = W * t
                ve = nc.vector
                nc.scalar.activation(dst[:, a:a + W], src[:, a:a + W], AF.Copy,
                                     scale=float(taps[2]))
                _stt(ve, dst[:, a + 2:a + W], src[:, a:a + W - 2], float(taps[0]),
                     dst[:, a + 2:a + W], op.mult, op.add)
                _stt(ve, dst[:, a + 1:a + W], src[:, a:a + W - 1], float(taps[1]),
                     dst[:, a + 1:a + W], op.mult, op.add)
                _stt(ve, dst[:, a:a + W - 1], src[:, a + 1:a + W], float(taps[3]),
                     dst[:, a:a + W - 1], op.mult, op.add)
                _stt(ve, dst[:, a:a + W - 2], src[:, a + 2:a + W], float(taps[4]),
                     dst[:, a:a + W - 2], op.mult, op.add)

        # g_full = g1 @ C on PE: per row-tile, transpose 128x128 subtiles of g1
        # (PE transpose mode), then banded matmuls against the C master strip.
        hcC = pool.tile([128, 2176], dt.float32, tag="Dt", name="hcC")
        nc.sync.dma_start(hcC[:], hc_d[:])
        GF = A              # gray dead; g_full lands in A
        def _hp(db, doff, sb, soff):
            # one 5-tap pass on a 1024 block: db[doff:] = BW(sb[soff:])
            nc.scalar.activation(db[:, doff:doff + W], sb[:, soff:soff + W],
                                 AF.Copy, scale=float(taps[2]))
            _stt(nc.vector, db[:, doff + 2:doff + W], sb[:, soff:soff + W - 2],
                 float(taps[0]), db[:, doff + 2:doff + W], op.mult, op.add)
            _stt(nc.vector, db[:, doff + 1:doff + W], sb[:, soff:soff + W - 1],
                 float(taps[1]), db[:, doff + 1:doff + W], op.mult, op.add)
            _stt(nc.vector, db[:, doff:doff + W - 1], sb[:, soff + 1:soff + W],
                 float(taps[3]), db[:, doff:doff + W - 1], op.mult, op.add)
            _stt(nc.vector, db[:, doff:doff + W - 2], sb[:, soff + 2:soff + W],
                 float(taps[4]), db[:, doff:doff + W - 2], op.mult, op.add)

        for t in range(4, NB):     # DVE path for blocks 4..7 (overlaps PE blocks)
            a = W * t
            tmp = pool.tile([128, W], dt.float32, tag="chG", name="htmp")
            _hp(tmp, 0, Bb, a)
            _hp(GF, a, tmp, 0)
        for t in range(4):
            g1T = pool.tile([128, 1024], dt.float32, tag="chR", name="g1T")
            for k in range(8):
                pst = psB.tile([128, 128], dt.float32, tag="mmD1", name="tp", bufs=2)
                nc.tensor.transpose(pst[:], Bb[:, W * t + 128 * k: W * t + 128 * (k + 1)],
                                    bmat2(EI['ident']))
                nc.scalar.activation(g1T[:, 128 * k:128 * (k + 1)], pst[:], AF.Copy)
            for n in range(2):
                pso = psA.tile([128, 512], dt.float32, tag="mmB", name="hco", bufs=2)
                vs = [0, 1, 2, 3, 4] if n == 0 else [3, 4, 5, 6, 7]
                for j, vv in enumerate(vs):
                    if (vv, n) == (0, 0):
                        rhs = hcC[:, 1152:1664]
                    elif (vv, n) == (7, 1):
                        rhs = hcC[:, 1664:2176]
                    else:
                        o = 512 + 512 * n - 128 * vv
                        rhs = hcC[:, o:o + 512]
                    nc.tensor.matmul(pso[:], g1T[:, 128 * vv:128 * (vv + 1)], rhs,
                                     start=(j == 0), stop=(j == len(vs) - 1))
                nc.scalar.activation(blk(GF, t, n), pso[:], AF.Copy)
        if debug:
            nc.sync.dma_start(dbg["gfull"][:], GF[:])

        # ---------------- sobel vertical parts: u/d row shifts on PE ---------
        # u[p] = g[p-1], d[p] = g[p+1]; s1 = u + 2g + d ; d1 = d - u (DVE)
        U = f32buf("Bt", "ush")      # g1 dead after hconv
        Dd = f32buf("Ct", "dsh")     # band1 dead
        for t in range(NB):
            for n in range(2):
                ps = psA.tile([128, 512], dt.float32, tag="mmS", name="mmS", bufs=2)
                nc.tensor.matmul(ps[:], bmat2(EI['shup']), blk(GF, t, n),
                                 start=True, stop=True)
                nc.scalar.activation(blk(U, t, n), ps[:], AF.Copy)
                ps2 = psB.tile([128, 512], dt.float32, tag="mmD1", name="mmD1", bufs=2)
                nc.tensor.matmul(ps2[:], bmat2(EI['shdn']), blk(GF, t, n),
                                 start=True, stop=True)
                nc.scalar.activation(blk(Dd, t, n), ps2[:], AF.Copy)
        # block-crossing boundary rows
        nc.sync.dma_start(U[0:1, W:FF], GF[127:128, 0:FF - W])
        nc.vector.memset(U[0:1, 0:W], 0.0)
        nc.scalar.dma_start(Dd[127:128, 0:FF - W], GF[0:1, W:FF])
        nc.scalar.dma_start(Dd[127:128, FF - W:FF], zf_d[:])
        D1 = f32buf("Dt", "d1")      # hcC dead
        nc.vector.tensor_tensor(D1[:], Dd[:], U[:], op.subtract)
        _stt(nc.vector, U[:], GF[:], 2.0, U[:], op.mult, op.add)
        nc.vector.tensor_tensor(U[:], U[:], Dd[:], op.add)
        S1 = U      # s1 now lives in Bt

        # gx = D_W(s1) -> Ct ; gy = S_W(d1) -> Et
        GX = Dd
        GY = f32buf("Et", "gy")
        for t in range(NB):
            a = W * t
            ve = nc.vector
            # gx[j] = s1[j+1] - s1[j-1]
            nc.scalar.activation(GX[:, a:a + W - 1], S1[:, a + 1:a + W], AF.Copy)
            nc.vector.memset(GX[:, a + W - 1:a + W], 0.0)
            _stt(ve, GX[:, a + 1:a + W], S1[:, a:a + W - 1], -1.0,
                 GX[:, a + 1:a + W], op.mult, op.add)
            # gy[j] = d1[j-1] + 2 d1[j] + d1[j+1]
            nc.scalar.activation(GY[:, a:a + W], D1[:, a:a + W], AF.Copy, scale=2.0)
            _stt(ve, GY[:, a + 1:a + W], D1[:, a:a + W - 1], 1.0,
                 GY[:, a + 1:a + W], op.mult, op.add)
            _stt(ve, GY[:, a:a + W - 1], D1[:, a + 1:a + W], 1.0,
                 GY[:, a:a + W - 1], op.mult, op.add)
        if debug:
            nc.sync.dma_start(dbg["gx"][:], GX[:])
            nc.sync.dma_start(dbg["gy"][:], GY[:])

        # ---------------- m2 / direction masks ------------------------------
        M2X = A             # g_full dead after sobel PE
        M2Y = D1            # d1 dead after gy
        nc.scalar.activation(M2X[:], GX[:], AF.Square)
        nc.scalar.activation(M2Y[:], GY[:], AF.Square)
        c0 = pool.tile([128, FF], dt.uint8, tag="haa", name="c0")
        c2 = pool.tile([128, FF], dt.uint8, tag="hcc", name="c2")
        c1 = pool.tile([128, FF], dt.uint8, tag="hbb", name="c1")
        _stt(nc.vector, c0[:], M2X[:], float(C['T1SQ']), M2Y[:], op.mult, op.is_ge)
        _stt(nc.vector, c2[:], M2X[:], float(C['T2SQ']), M2Y[:], op.mult, op.is_le)
        M2 = S1             # s1 dead after gx (Bt slot)
        nc.gpsimd.tensor_tensor(M2[:], M2X[:], M2Y[:], op.add)
        PROD = GX           # overwrite gx in place
        nc.vector.tensor_tensor(PROD[:], GX[:], GY[:], op.mult)
        # c1 = (prod >= 0) & !(c0 | c2)
        nc.vector.tensor_tensor(c1[:], c0[:], c2[:], op.logical_or)
        nc.vector.tensor_scalar(c1[:], c1[:], 0.0, None, op.is_equal)
        _stt(nc.vector, c1[:], PROD[:], 0.0, c1[:], op.is_ge, op.logical_and)
        if debug:
            nc.sync.dma_start(dbg["m2"][:], M2[:])

        # ---------------- NMS ------------------------------------------------
        # m2u = m2[row-1] -> M2X slot ; m2d = m2[row+1] -> M2Y slot (PE shifts)
        M2U = A
        M2D = D1
        for t in range(NB):
            for n in range(2):
                ps = psA.tile([128, 512], dt.float32, tag="mmU", name="mmU", bufs=1)
                nc.tensor.matmul(ps[:], bmat2(EI['shup']), blk(M2, t, n),
                                 start=True, stop=True)
                nc.scalar.activation(blk(M2U, t, n), ps[:], AF.Copy)
                ps2 = psB.tile([128, 512], dt.float32, tag="mmV", name="mmV", bufs=1)
                nc.tensor.matmul(ps2[:], bmat2(EI['shdn']), blk(M2, t, n),
                                 start=True, stop=True)
                nc.scalar.activation(blk(M2D, t, n), ps2[:], AF.Copy)
        # boundary rows across blocks
        nc.sync.dma_start(M2U[0:1, W:FF], M2[127:128, 0:FF - W])
        nc.vector.memset(M2U[0:1, 0:W], 0.0)
        nc.scalar.dma_start(M2D[127:128, 0:FF - W], M2[0:1, W:FF])
        nc.scalar.dma_start(M2D[127:128, FF - W:FF], zf_d[:])

        km = GY             # gy dead after prod
        scrtags = ["chR", "chG", "chB"]

        def nms_cat(catmask, a1, sh1, a2, sh2, first, ci):
            for t in range(NB):
                a = W * t
                lo = max(0, -sh1, -sh2)
                hi = min(W, W - sh1, W - sh2)
                tg = scrtags[(ci * NB + t) % 3]
                eng = nc.vector
                scr = pool.tile([128, W], dt.float32, tag=tg, name="scr")
                eng.tensor_tensor(scr[:, lo:hi],
                                  a1[:, a + lo + sh1:a + hi + sh1],
                                  a2[:, a + lo + sh2:a + hi + sh2], op.max)
                eng.tensor_tensor(scr[:, lo:hi], M2[:, a + lo:a + hi],
                                  scr[:, lo:hi], op.is_ge)
                if first:
                    nc.vector.tensor_copy(km[:, a + lo:a + hi], scr[:, lo:hi])
                    if lo > 0:
                        nc.vector.memset(km[:, a:a + lo], 0.0)
                        nc.vector.memset(km[:, a + hi:a + W], 0.0)
                else:
                    nc.vector.copy_predicated(km[:, a + lo:a + hi],
                                              catmask[:, a + lo:a + hi],
                                              scr[:, lo:hi])

        nms_cat(None, M2U, -1, M2D, 1, True, 0)     # cat3
        nms_cat(c2, M2U, 0, M2D, 0, False, 1)       # cat2
        nms_cat(c1, M2U, 1, M2D, -1, False, 2)      # cat1
        nms_cat(c0, M2, -1, M2, 1, False, 3)        # cat0
        if debug:
            nc.sync.dma_start(dbg["kmask"][:], km[:])

        # ---------------- thresholds -> f32 0/1 ------------------------------
        lowf = GX            # prod dead
        e0f = M2D            # m2d dead after cat1
        _stt(nc.vector, lowf[:], M2[:], float(C['KLOW']), km[:], op.is_gt, op.mult)
        _stt(nc.vector, e0f[:], M2[:], float(C['KHIGH']), km[:], op.is_gt, op.mult)
        if debug:
            nc.sync.dma_start(dbg["low0"][:], lowf[:])
            nc.sync.dma_start(dbg["e0"][:], e0f[:])

        # ---------------- pack to bits ---------------------------------------
        pia = pool.tile([128, FF], dt.int32, tag="A", name="pia")
        pib = pool.tile([128, FF], dt.int32, tag="Et", name="pib")

        def pack(dstp, srcf):
            nc.vector.tensor_copy(pia[:], srcf[:])          # f32 0/1 -> int32
            cur = pia[:]
            n = FF
            for lvl in range(5):
                n //= 2
                sh = 1 << lvl
                vw = cur.rearrange("p (n two) -> p n two", two=2)
                if lvl == 4:
                    dst = dstp[:]
                elif lvl % 2 == 0:
                    dst = pib[:, 0:n]
                else:
                    dst = pia[:, 0:n]
                _stt(nc.vector, dst, vw[:, :, 1], sh, vw[:, :, 0],
                     op.logical_shift_left, op.bitwise_or)
                cur = dst

        lowp = pool.tile([128, PF], dt.int32, tag="lp2", name="lowp")
        e0p = pool.tile([128, PF], dt.int32, tag="ep2", name="e0p")
        pack(lowp, lowf)
        pack(e0p, e0f)
        nc.vector.tensor_tensor(lowp[:], lowp[:], ipm[:], op.bitwise_and)
        nc.vector.tensor_tensor(e0p[:], e0p[:], ipm[:], op.bitwise_and)
        if debug:
            nc.sync.dma_start(dbg["lowp"][:], lowp[:])
            nc.sync.dma_start(dbg["e0p"][:], e0p[:])

        # ---------------- hysteresis flood fill ------------------------------
        e = e0p
        aa = pool.tile([128, PF], dt.int32, tag="haa", name="haa")
        bb2 = pool.tile([128, PF], dt.int32, tag="hbb", name="hbb")
        cc = pool.tile([128, PF], dt.int32, tag="hcc", name="hcc")
        dup = pool.tile([128, PF], dt.int32, tag="hdup", name="hdup")
        ddn = pool.tile([128, PF], dt.int32, tag="hddn", name="hddn")
        av = aa.rearrange("p (n w) -> p n w", w=PW)
        bv = bb2.rearrange("p (n w) -> p n w", w=PW)
        cv = cc.rearrange("p (n w) -> p n w", w=PW)
        mask_up = [min(i + 1, 31) for i in range(32)]   # dup[p] = c[p+1]
        mask_dn = [max(i - 1, 0) for i in range(32)]    # ddn[p] = c[p-1]
        for it in range(N_ITER):
            # horizontal dilate with cross-word carries
            _stt(nc.vector, aa[:], e[:], 1, e[:], op.logical_shift_left, op.bitwise_or)
            _stt(nc.vector, aa[:], e[:], 1, aa[:], op.logical_shift_right, op.bitwise_or)
            _stt(nc.vector, bb2[:, 1:PF], e[:, 0:PF - 1], 31, aa[:, 1:PF],
                 op.logical_shift_right, op.bitwise_or)
            nc.vector.tensor_copy(bv[:, :, 0], av[:, :, 0])
            _stt(nc.vector, cc[:, 0:PF - 1], e[:, 1:PF], 31, bb2[:, 0:PF - 1],
                 op.logical_shift_left, op.bitwise_or)
            nc.vector.tensor_copy(cv[:, :, PW - 1], bv[:, :, PW - 1])
            # vertical neighbors via stream_shuffle + boundary DMAs
            nc.vector.stream_shuffle(dup[:], cc[:], mask_up)
            nc.vector.stream_shuffle(ddn[:], cc[:], mask_dn)
            q1 = nc.sync if it % 2 == 0 else nc.scalar
            q2 = nc.scalar if it % 2 == 0 else nc.sync
            # rows masked to zero by lowp's interior mask need no zero fill;
            # each direction's two fixes go to different queues to parallelize
            q1.dma_start(dup[31:127:32, :], cc[32:128:32, :])
            q2.dma_start(dup[127:128, 0:PF - PW], cc[0:1, PW:PF])
            q1.dma_start(ddn[32:128:32, :], cc[31:127:32, :])
            q2.dma_start(ddn[0:1, PW:PF], cc[127:128, 0:PF - PW])
            # e' = (dup | ddn | c) & low
            nc.vector.tensor_tensor(dup[:], dup[:], ddn[:], op.bitwise_or)
            nc.vector.tensor_tensor(dup[:], dup[:], cc[:], op.bitwise_or)
            nc.vector.tensor_tensor(e[:], dup[:], lowp[:], op.bitwise_and)
        if debug:
            nc.sync.dma_start(dbg["ep"][:], e[:])

        # ---------------- unpack complement -> output ------------------------
        ne = aa
        _stt(nc.vector, ne[:], e[:], -1, e[:], op.bitwise_xor, op.bypass)
        outf = f32buf("A", "outf")
        cur = ne[:]
        n = PF
        masks = [0xFFFF, 0xFF, 0xF, 0x3, 0x1]
        for lvl in range(5):
            half = 32 >> (lvl + 1)       # bits per half-word after split
            m = masks[lvl]
            dst = (pib[:, 0:2 * n] if lvl % 2 == 0 else pia[:, 0:2 * n])
            dv2 = dst.rearrange("p (n two) -> p n two", two=2)
            _ts_int(nc.vector, dv2[:, :, 0], cur, m, op.bitwise_and)
            _ts_int(nc.vector, dv2[:, :, 1], cur, half, op.logical_shift_right,
                    m, op.bitwise_and)
            cur = dst
            n *= 2
        nc.vector.tensor_copy(outf[:], pib[:])    # int 0/1 -> f32
        for t in range(NB):
            q = nc.sync if t % 2 == 0 else nc.scalar
            q.dma_start(out_d[128 * t:128 * (t + 1), :], outf[:, W * t:W * (t + 1)])

    nc.compile()
    return nc, C, dbg


def _run(inputs, debug=False, trace=False):
    from concourse.bass_utils import run_bass_kernel_spmd
    key = ("dbg" if debug else "plain")
    if key not in _cache:
        _cache[key] = build_program(debug=debug)
    nc, C, dbg = _cache[key]
    x = np.asarray(inputs["x"], dtype=np.float32)
    in_maps = []
    for c in range(B):
        in_maps.append({
            "x": np.ascontiguousarray(x[c]),
            "bandT": C['bandT'],
            "band2T": C['band2T'],
            "hcC": C['hcC'],
            "interior": C['interior_packed'],
            "zeros_f32": C['zeros_f32'],
            "zeros_i32": C['zeros_i32'],
        })
    res = run_bass_kernel_spmd(nc, in_maps, core_ids=list(range(B)), trace=trace)
    return res


def kernel(x, gaussian_kernel=None, sobel_x=None, sobel_y=None):
    res = _run({"x": x})
    out = np.stack([res.results[c]["out"] for c in range(B)], axis=0)
    return out.reshape(B, 1, H, W).astype(np.float32)
